# revision 1
# baseline (speedup 1.0000x reference)
"""GCN encoder (2-layer) Bass kernel for Trainium2, 8 NeuronCores.

Strategy (graph/data parallel, per sharding hint):
  - Nodes padded to NPAD=50176 and sharded by contiguous range: core c owns
    destination nodes [c*6272, (c+1)*6272) = 49 blocks of 128.
  - Edges (incl. self-loops) are bucketed by destination block and by source
    half (dma_gather indices are int16, so the feature table is gathered in
    two halves of 25088 rows each). Every (block, half) bucket is padded to a
    uniform tile count TH so all 8 cores run one identical SPMD program.
  - Per layer: h = x @ W (dense matmul, PSUM f32), table hs = h * dinv[src]
    stored in HBM (bf16); per destination block, edge messages are fetched
    with dma_gather (128 edges/tile, edge-major) and segment-summed on the
    TensorEngine via one-hot matmuls: onehot[k,d] = w[k] * (col[k]==d), so
    PSUM[d,f] += sum_k w[k]*hs[src_k][f]. Post: * dinv[dest] + bias (+relu).
  - Layer boundary: hs2 shards are exchanged with an AllGather collective.
  - deg = padded per-node weight lists reduced on DVE; dinv = sqrt(1/deg).

kernel(**inputs) takes the FULL inputs and returns the FULL [50000,128] f32
output; all sharding/gather happens inside.
"""

import sys

sys.path.insert(0, "/opt/trn_rl_repo")

import numpy as np
import ml_dtypes

P = 128
NCORES = 8
BPC = 49                 # dest blocks per core
SHARD = BPC * P          # 6272
NPAD = NCORES * SHARD    # 50176
HALF = NPAD // 2         # 25088
N = 50000
FIN = 256
H = 256                  # layer-1 output width
F2 = 128                 # layer-2 output width
DUMMY_SRC = N + 8        # a zero (pad) node, used as src for pad edges

_BF16 = ml_dtypes.bfloat16


def _preprocess(edge_index, edge_weight):
    """Build all per-core device input arrays from the edge list."""
    row = np.asarray(edge_index[0], dtype=np.int64)
    col = np.asarray(edge_index[1], dtype=np.int64)
    w = np.asarray(edge_weight, dtype=np.float32)

    loop = np.arange(N, dtype=np.int64)
    rows = np.concatenate([row, loop])
    cols = np.concatenate([col, loop])
    ws = np.concatenate([w, np.ones(N, np.float32)])
    EE = rows.shape[0]

    # ---- per-node padded weight lists (for deg on device) ----
    deg_cnt = np.bincount(cols, minlength=NPAD)
    L = int(deg_cnt.max())
    L = (L + 7) & ~7  # round to multiple of 8
    order = np.argsort(cols, kind="stable")
    cs = np.zeros(NPAD + 1, np.int64)
    np.cumsum(deg_cnt, out=cs[1:])
    slot = np.arange(EE) - cs[cols[order]]
    wdeg = np.zeros((NPAD, L), np.float32)
    wdeg[cols[order], slot] = ws[order]
    # partition-major: wdegP[p, nb*L+l] = wdeg[nb*128+p, l]
    wdegP = np.ascontiguousarray(
        wdeg.reshape(NPAD // P, P, L).transpose(1, 0, 2).reshape(P, (NPAD // P) * L)
    )

    # ---- edge streams per (block, half) ----
    blk = cols // P                      # 0..390 (real dests only)
    half = (rows >= HALF).astype(np.int64)
    key = blk * 2 + half
    cnt = np.bincount(key, minlength=(NPAD // P) * 2)
    TH = int(-(-cnt.max() // P))         # tiles per half
    CAP = TH * P
    NB = NPAD // P                       # 392 blocks

    src_a = np.full((NB, 2, CAP), DUMMY_SRC % HALF, np.int16)
    col_a = np.zeros((NB, 2, CAP), np.float32)
    w_a = np.zeros((NB, 2, CAP), np.float32)

    order2 = np.argsort(key, kind="stable")
    cs2 = np.zeros(NB * 2 + 1, np.int64)
    np.cumsum(cnt, out=cs2[1:])
    pos = np.arange(EE) - cs2[key[order2]]
    kb = key[order2] // 2
    kh = key[order2] % 2
    src_sorted = rows[order2]
    src_rel = np.where(kh == 1, src_sorted - HALF, src_sorted).astype(np.int16)
    src_a[kb, kh, pos] = src_rel
    col_a[kb, kh, pos] = (cols[order2] - kb * P).astype(np.float32)
    w_a[kb, kh, pos] = ws[order2]

    # wrapped int16 index layout for dma_gather: index i -> partition i%16,
    # col i//16, replicated across the 8 groups of 16 partitions.
    IW = CAP // 16
    idx_w = src_a.reshape(NB, 2, IW, 16).transpose(0, 1, 3, 2)  # [NB,2,16,IW]
    idx_w = np.ascontiguousarray(np.tile(idx_w, (1, 1, 8, 1)))  # [NB,2,128,IW]

    # col/w in per-tile scalar layout: [.., 128, 2*TH] where slot (h*TH+t)
    # on partition p = edge t*128+p of half h.
    colP = col_a.reshape(NB, 2, TH, P).transpose(3, 0, 1, 2).reshape(P, NB * 2 * TH)
    wfP = w_a.reshape(NB, 2, TH, P).transpose(3, 0, 1, 2).reshape(P, NB * 2 * TH)
    colP = np.ascontiguousarray(colP)
    wfP = np.ascontiguousarray(wfP)

    return dict(L=L, TH=TH, CAP=CAP, wdegP=wdegP, idx_w=idx_w, colP=colP, wfP=wfP)


def _host_golden(x, W1, b1, W2, b2, pp, out_dtype=np.float32, quant=True):
    """Numpy re-implementation of the exact device algorithm (same tiling,
    same bf16 quantization points). For validating the scheme off-device."""
    bf = (lambda a: a.astype(_BF16).astype(np.float32)) if quant else (lambda a: a)
    TH, CAP, L = pp["TH"], pp["CAP"], pp["L"]
    NB = NPAD // P

    wdegP = pp["wdegP"]
    deg = np.zeros(NPAD, np.float32)
    for nb in range(NB):
        blkw = wdegP[:, nb * L:(nb + 1) * L]
        deg[nb * P:(nb + 1) * P] = blkw.sum(axis=1)
    dinv = np.sqrt(1.0 / (deg + (deg == 0)))

    xp = np.zeros((NPAD, FIN), np.float32)
    xp[:N] = x
    h1 = bf(xp) @ bf(W1)                     # bf16 inputs, f32 accum
    hs1 = bf(h1 * dinv[:, None])             # stored bf16

    idx_w = pp["idx_w"]; colP = pp["colP"]; wfP = pp["wfP"]
    out1 = np.zeros((NPAD, H), np.float32)
    for nb in range(NB):
        acc = np.zeros((P, H), np.float32)
        for hh in range(2):
            iw = idx_w[nb, hh, :16, :]                      # [16, IW]
            flat = iw.T.reshape(-1)[:CAP].astype(np.int64)  # unwrap
            base = 0 if hh == 0 else HALF
            msgs = hs1[base + flat]                         # [CAP, H]
            for t in range(TH):
                oh = np.zeros((P, P), np.float32)
                c = colP[:, (nb * 2 + hh) * TH + t]
                wv = bf(wfP[:, (nb * 2 + hh) * TH + t])
                oh[np.arange(P), c.astype(np.int64)] = wv
                acc += oh.T @ msgs[t * P:(t + 1) * P]
        z = acc * dinv[nb * P:(nb + 1) * P, None] + b1[None, :]
        out1[nb * P:(nb + 1) * P] = np.maximum(z, 0.0)

    h2in = bf(out1)
    h2 = h2in @ bf(W2)
    hs2 = bf(h2 * dinv[:, None])

    out2 = np.zeros((NPAD, F2), np.float32)
    for nb in range(NB):
        acc = np.zeros((P, F2), np.float32)
        for hh in range(2):
            iw = idx_w[nb, hh, :16, :]
            flat = iw.T.reshape(-1)[:CAP].astype(np.int64)
            base = 0 if hh == 0 else HALF
            msgs = hs2[base + flat]
            for t in range(TH):
                oh = np.zeros((P, P), np.float32)
                c = colP[:, (nb * 2 + hh) * TH + t]
                wv = bf(wfP[:, (nb * 2 + hh) * TH + t])
                oh[np.arange(P), c.astype(np.int64)] = wv
                acc += oh.T @ msgs[t * P:(t + 1) * P]
        out2[nb * P:(nb + 1) * P] = (
            acc * dinv[nb * P:(nb + 1) * P, None] + b2[None, :]
        )
    return out2[:N].astype(out_dtype)


# ---------------------------------------------------------------------------
# Bass device kernel
# ---------------------------------------------------------------------------

_NC_CACHE = {}


def _build_nc(TH, L):
    import concourse.bass as bass  # noqa: F401
    import concourse.mybir as mybir
    import concourse.tile as tile
    from concourse import bacc
    from concourse.library_config import mlp

    DT = mybir.dt.bfloat16
    F32 = mybir.dt.float32
    I16 = mybir.dt.int16
    AL = mybir.AluOpType
    AF = mybir.ActivationFunctionType
    AX = mybir.AxisListType

    CAP = TH * P
    IW = CAP // 16
    NB = NPAD // P           # 392
    NBC = 56                 # wdeg chunk: blocks per chunk (392 = 7*56)

    nc = bacc.Bacc("TRN2", target_bir_lowering=False, debug=True,
                   num_devices=NCORES)
    xt3_d = nc.dram_tensor("xt3", [2, P, NPAD], DT, kind="ExternalInput")
    w1_d = nc.dram_tensor("w1c", [2, P, H], DT, kind="ExternalInput")
    w2_d = nc.dram_tensor("w2c", [2, P, F2], DT, kind="ExternalInput")
    b1_d = nc.dram_tensor("b1f", [P, H], F32, kind="ExternalInput")
    b2_d = nc.dram_tensor("b2f", [P, F2], F32, kind="ExternalInput")
    iota_d = nc.dram_tensor("iota", [P, P], F32, kind="ExternalInput")
    wdeg_d = nc.dram_tensor("wdegP", [P, NB * L], F32, kind="ExternalInput")
    wdegl_d = nc.dram_tensor("wdeglP", [P, BPC * L], F32, kind="ExternalInput")
    idx_d = nc.dram_tensor("idxP", [P, BPC * 2 * IW], I16, kind="ExternalInput")
    col_d = nc.dram_tensor("colP", [P, BPC * 2 * TH], F32, kind="ExternalInput")
    wf_d = nc.dram_tensor("wfP", [P, BPC * 2 * TH], F32, kind="ExternalInput")
    out_d = nc.dram_tensor("out2", [SHARD, F2], F32, kind="ExternalOutput")

    with tile.TileContext(nc) as tc:
        with (
            tc.tile_pool(name="dram", bufs=1, space="DRAM") as dpool,
            tc.tile_pool(name="const", bufs=1) as cpool,
            tc.tile_pool(name="wdegc", bufs=2) as wpool,
            tc.tile_pool(name="xs", bufs=3) as xpool,
            tc.tile_pool(name="hst", bufs=3) as hpool,
            tc.tile_pool(name="msg", bufs=2) as mpool,
            tc.tile_pool(name="oh", bufs=8) as ohpool,
            tc.tile_pool(name="post", bufs=3) as tpool,
            tc.tile_pool(name="ph1", bufs=2, space="PSUM") as ph1p,
            tc.tile_pool(name="pagg", bufs=2, space="PSUM") as paggp,
            tc.tile_pool(name="pc", bufs=2, space="PSUM") as pcp,
        ):
            hs1_tab = dpool.tile([NPAD, H], DT)
            h2in_dram = dpool.tile([SHARD, H], DT)
            hs2_shard = dpool.tile([SHARD, F2], DT)
            hs2_full = dpool.tile([NPAD, F2], DT, addr_space="Shared")

            nc.gpsimd.load_library(mlp)

            # ---- constants ----
            w1_sb = cpool.tile([P, 2 * H], DT)
            nc.sync.dma_start(out=w1_sb[:, 0:H], in_=w1_d[0])
            nc.sync.dma_start(out=w1_sb[:, H:2 * H], in_=w1_d[1])
            w2_sb = cpool.tile([P, 2 * F2], DT)
            nc.sync.dma_start(out=w2_sb[:, 0:F2], in_=w2_d[0])
            nc.sync.dma_start(out=w2_sb[:, F2:2 * F2], in_=w2_d[1])
            b1_sb = cpool.tile([P, H], F32)
            nc.sync.dma_start(out=b1_sb[:], in_=b1_d[:])
            b2_sb = cpool.tile([P, F2], F32)
            nc.sync.dma_start(out=b2_sb[:], in_=b2_d[:])
            iota_sb = cpool.tile([P, P], F32)
            nc.sync.dma_start(out=iota_sb[:], in_=iota_d[:])
            idx_sb = cpool.tile([P, BPC * 2 * IW], I16)
            nc.sync.dma_start(out=idx_sb[:], in_=idx_d[:])
            col_sb = cpool.tile([P, BPC * 2 * TH], F32)
            nc.sync.dma_start(out=col_sb[:], in_=col_d[:])
            wf_sb = cpool.tile([P, BPC * 2 * TH], F32)
            nc.sync.dma_start(out=wf_sb[:], in_=wf_d[:])

            # ---- deg -> dinv (full, and local shard) ----
            deg_sb = cpool.tile([P, NB], F32)
            for ch in range(NB // NBC):
                wt = wpool.tile([P, NBC * L], F32, tag="wdeg")
                nc.sync.dma_start(out=wt[:], in_=wdeg_d[:, ch * NBC * L:(ch + 1) * NBC * L])
                nc.vector.reduce_sum(
                    deg_sb[:, ch * NBC:(ch + 1) * NBC],
                    wt[:].rearrange("p (nb l) -> p nb l", l=L),
                    axis=AX.X,
                )
            eq_sb = cpool.tile([P, NB], F32)
            nc.vector.tensor_scalar(eq_sb[:], deg_sb[:], 0.0, None, AL.is_equal)
            nc.vector.tensor_tensor(deg_sb[:], deg_sb[:], eq_sb[:], AL.add)
            rec_sb = cpool.tile([P, NB], F32)
            nc.vector.reciprocal(rec_sb[:], deg_sb[:])
            dinv_sb = cpool.tile([P, NB], F32)
            nc.scalar.sqrt(dinv_sb[:], rec_sb[:])

            wl_sb = cpool.tile([P, BPC * L], F32)
            nc.sync.dma_start(out=wl_sb[:], in_=wdegl_d[:])
            degl_sb = cpool.tile([P, BPC], F32)
            nc.vector.reduce_sum(
                degl_sb[:], wl_sb[:].rearrange("p (nb l) -> p nb l", l=L), axis=AX.X
            )
            eql_sb = cpool.tile([P, BPC], F32)
            nc.vector.tensor_scalar(eql_sb[:], degl_sb[:], 0.0, None, AL.is_equal)
            nc.vector.tensor_tensor(degl_sb[:], degl_sb[:], eql_sb[:], AL.add)
            recl_sb = cpool.tile([P, BPC], F32)
            nc.vector.reciprocal(recl_sb[:], degl_sb[:])
            dinvl_sb = cpool.tile([P, BPC], F32)
            nc.scalar.sqrt(dinvl_sb[:], recl_sb[:])

            # ---- phase A: h1 = x @ W1 (all nodes), hs1 = h1 * dinv ----
            for s in range(NPAD // 512):
                xa = xpool.tile([P, 512], DT, tag="xa")
                xb = xpool.tile([P, 512], DT, tag="xb")
                nc.sync.dma_start(out=xa[:], in_=xt3_d[0][:, s * 512:(s + 1) * 512])
                nc.sync.dma_start(out=xb[:], in_=xt3_d[1][:, s * 512:(s + 1) * 512])
                for q in range(4):
                    nb = s * 4 + q
                    ph = ph1p.tile([P, H], F32)
                    nc.tensor.matmul(ph[:], lhsT=xa[:, q * P:(q + 1) * P],
                                     rhs=w1_sb[:, 0:H], start=True, stop=False)
                    nc.tensor.matmul(ph[:], lhsT=xb[:, q * P:(q + 1) * P],
                                     rhs=w1_sb[:, H:2 * H], start=False, stop=True)
                    hst = hpool.tile([P, H], DT, tag="hst")
                    nc.scalar.activation(hst[:], ph[:], AF.Copy,
                                         scale=dinv_sb[:, nb:nb + 1])
                    nc.sync.dma_start(out=hs1_tab[nb * P:(nb + 1) * P, :], in_=hst[:])

            # ---- phase B: layer-1 aggregation per dest block ----
            for b in range(BPC):
                msgs = []
                for hh in range(2):
                    m = mpool.tile([P, TH, H], DT, tag=f"msg{hh}")
                    src = hs1_tab[0:HALF, :] if hh == 0 else hs1_tab[HALF:NPAD, :]
                    nc.gpsimd.dma_gather(
                        m[:], src, idx_sb[:, (b * 2 + hh) * IW:(b * 2 + hh + 1) * IW],
                        CAP, CAP, H, single_packet=False)
                    msgs.append(m)
                pagg = paggp.tile([P, H], F32)
                for t in range(2 * TH):
                    hh, tt = (0, t) if t < TH else (1, t - TH)
                    oh = ohpool.tile([P, P], DT, tag="oh")
                    sc = (b * 2 + hh) * TH + tt
                    nc.vector.tensor_scalar(oh[:], iota_sb[:], col_sb[:, sc:sc + 1],
                                            wf_sb[:, sc:sc + 1], AL.is_equal, AL.mult)
                    nc.tensor.matmul(pagg[:], lhsT=oh[:], rhs=msgs[hh][:, tt, :],
                                     start=(t == 0), stop=(t == 2 * TH - 1))
                t1 = tpool.tile([P, H], F32, tag="t1")
                nc.vector.tensor_scalar(t1[:], pagg[:], dinvl_sb[:, b:b + 1], None,
                                        AL.mult)
                t2 = tpool.tile([P, H], F32, tag="t2")
                nc.vector.tensor_tensor(t2[:], t1[:], b1_sb[:], AL.add)
                rl = hpool.tile([P, H], DT, tag="rl")
                nc.scalar.activation(rl[:], t2[:], AF.Relu)
                nc.sync.dma_start(out=h2in_dram[b * P:(b + 1) * P, :], in_=rl[:])

            # ---- phase C: h2 = relu_out @ W2, hs2 = h2 * dinv (own shard) ----
            for b in range(BPC):
                ph2 = pcp.tile([P, F2], F32, tag="pc")
                for c2 in range(2):
                    at = ohpool.tile([P, P], DT, tag="at")
                    nc.sync.dma_start(
                        out=at[:],
                        in_=h2in_dram[b * P:(b + 1) * P, c2 * P:(c2 + 1) * P],
                        transpose=True)
                    nc.tensor.matmul(ph2[:], lhsT=at[:],
                                     rhs=w2_sb[:, c2 * F2:(c2 + 1) * F2],
                                     start=(c2 == 0), stop=(c2 == 1))
                hsb = hpool.tile([P, F2], DT, tag="hsb")
                nc.scalar.activation(hsb[:], ph2[:], AF.Copy,
                                     scale=dinvl_sb[:, b:b + 1])
                nc.sync.dma_start(out=hs2_shard[b * P:(b + 1) * P, :], in_=hsb[:])

            # ---- phase D: exchange hs2 shards ----
            nc.gpsimd.collective_compute(
                "AllGather", AL.bypass,
                replica_groups=[list(range(NCORES))],
                ins=[hs2_shard[:]],
                outs=[hs2_full[:]],
            )

            # ---- phase E: layer-2 aggregation per dest block ----
            for b in range(BPC):
                msgs = []
                for hh in range(2):
                    m = mpool.tile([P, TH, F2], DT, tag=f"msg{hh}")
                    src = hs2_full[0:HALF, :] if hh == 0 else hs2_full[HALF:NPAD, :]
                    nc.gpsimd.dma_gather(
                        m[:], src, idx_sb[:, (b * 2 + hh) * IW:(b * 2 + hh + 1) * IW],
                        CAP, CAP, F2, single_packet=False)
                    msgs.append(m)
                pagg2 = pcp.tile([P, F2], F32, tag="pc")
                for t in range(2 * TH):
                    hh, tt = (0, t) if t < TH else (1, t - TH)
                    oh = ohpool.tile([P, P], DT, tag="oh")
                    sc = (b * 2 + hh) * TH + tt
                    nc.vector.tensor_scalar(oh[:], iota_sb[:], col_sb[:, sc:sc + 1],
                                            wf_sb[:, sc:sc + 1], AL.is_equal, AL.mult)
                    nc.tensor.matmul(pagg2[:], lhsT=oh[:], rhs=msgs[hh][:, tt, :],
                                     start=(t == 0), stop=(t == 2 * TH - 1))
                o1 = tpool.tile([P, F2], F32, tag="o1")
                nc.vector.tensor_scalar(o1[:], pagg2[:], dinvl_sb[:, b:b + 1], None,
                                        AL.mult)
                o2 = tpool.tile([P, F2], F32, tag="o2")
                nc.vector.tensor_tensor(o2[:], o1[:], b2_sb[:], AL.add)
                nc.sync.dma_start(out=out_d[b * P:(b + 1) * P, :], in_=o2[:])

    nc.compile()
    return nc


def _make_inputs(x, W1, b1, W2, b2, pp):
    """Per-core input maps."""
    TH, L = pp["TH"], pp["L"]
    IW = (TH * P) // 16
    NB = NPAD // P

    xp = np.zeros((NPAD, FIN), np.float32)
    xp[:N] = x
    xt3 = np.ascontiguousarray(
        xp.T.reshape(2, P, NPAD).astype(_BF16))
    w1c = np.ascontiguousarray(W1.reshape(2, P, H).astype(_BF16))
    w2c = np.ascontiguousarray(W2.reshape(2, P, F2).astype(_BF16))
    b1f = np.ascontiguousarray(np.tile(b1[None, :], (P, 1)).astype(np.float32))
    b2f = np.ascontiguousarray(np.tile(b2[None, :], (P, 1)).astype(np.float32))
    iota = np.tile(np.arange(P, dtype=np.float32)[None, :], (P, 1))

    wdegP = pp["wdegP"]
    idx_w = pp["idx_w"]        # [NB, 2, 128, IW]
    colP = pp["colP"]          # [128, NB*2*TH]
    wfP = pp["wfP"]

    in_maps = []
    for c in range(NCORES):
        b0 = c * BPC
        idxP = np.ascontiguousarray(
            idx_w[b0:b0 + BPC].transpose(2, 0, 1, 3).reshape(P, BPC * 2 * IW))
        in_maps.append({
            "xt3": xt3,
            "w1c": w1c,
            "w2c": w2c,
            "b1f": b1f,
            "b2f": b2f,
            "iota": iota,
            "wdegP": wdegP,
            "wdeglP": np.ascontiguousarray(wdegP[:, b0 * L:(b0 + BPC) * L]),
            "idxP": idxP,
            "colP": np.ascontiguousarray(colP[:, b0 * 2 * TH:(b0 + BPC) * 2 * TH]),
            "wfP": np.ascontiguousarray(wfP[:, b0 * 2 * TH:(b0 + BPC) * 2 * TH]),
        })
    return in_maps


def kernel(x, edge_index, edge_weight, W1, b1, W2, b2, _trace=False):
    from concourse.bass_utils import run_bass_kernel_spmd

    x = np.asarray(x, dtype=np.float32)
    W1 = np.asarray(W1, dtype=np.float32)
    b1 = np.asarray(b1, dtype=np.float32)
    W2 = np.asarray(W2, dtype=np.float32)
    b2 = np.asarray(b2, dtype=np.float32)

    pp = _preprocess(np.asarray(edge_index), np.asarray(edge_weight))
    key = (pp["TH"], pp["L"])
    if key not in _NC_CACHE:
        _NC_CACHE[key] = _build_nc(*key)
    nc = _NC_CACHE[key]

    in_maps = _make_inputs(x, W1, b1, W2, b2, pp)
    res = run_bass_kernel_spmd(nc, in_maps, list(range(NCORES)), trace=_trace)
    out = np.concatenate([res.results[c]["out2"] for c in range(NCORES)], axis=0)
    if _trace:
        kernel._last_result = res
    return np.ascontiguousarray(out[:N])



# revision 8
# speedup vs baseline: 1.0399x; 1.0399x over previous
"""GCN encoder (2-layer) Bass kernel for Trainium2, 8 NeuronCores.

Strategy (graph/data parallel, per sharding hint):
  - Nodes padded to NPAD=50176 and sharded by contiguous range: core c owns
    destination nodes [c*6272, (c+1)*6272) = 49 blocks of 128.
  - Edges (incl. self-loops) are bucketed by destination block and by source
    half (dma_gather indices are int16, so the feature table is gathered in
    two halves of 25088 rows each), sorted by source within each bucket for
    HBM page locality. Buckets are padded to a uniform tile count TH so all 8
    cores run one identical SPMD program; trailing pad slots are -1 indices,
    which the gather ucode drops (descriptors stop at the real count).
  - Per layer: h = x @ W (dense matmul, PSUM f32), table hs = h * dinv[src]
    stored in HBM (bf16); per destination block, edge messages are fetched
    with dma_gather (128 edges/tile, edge-major, queue_num rotating over 4
    SWDGE queues) and segment-summed on the TensorEngine via one-hot matmuls:
    onehot[k,d] = w[k] * (col[k]==d), so PSUM[d,f] += sum_k w[k]*hs[src_k][f].
    Post: * dinv[dest] + bias (+relu). One-hots are built on DVE in pure bf16.
  - The layer-2 projection (relu_out @ W2) is interleaved into the layer-1
    block loop; hs2 shards are then exchanged with an AllGather collective.
  - dinv = rsqrt(deg) is precomputed on the host (graph-structure preproc).

kernel(**inputs) takes the FULL inputs and returns the FULL [50000,128] f32
output; all sharding/gather happens inside.
"""

import sys

sys.path.insert(0, "/opt/trn_rl_repo")

import numpy as np
import ml_dtypes

P = 128
NCORES = 8
BPC = 49                 # dest blocks per core
SHARD = BPC * P          # 6272
NPAD = NCORES * SHARD    # 50176
HALF = NPAD // 2         # 25088
N = 50000
FIN = 256
H = 256                  # layer-1 output width
F2 = 128                 # layer-2 output width
DUMMY_SRC = N + 8        # a zero (pad) node, used as src for pad edges

_BF16 = ml_dtypes.bfloat16


def _preprocess(edge_index, edge_weight):
    """Build all per-core device input arrays from the edge list."""
    row = np.asarray(edge_index[0], dtype=np.int64)
    col = np.asarray(edge_index[1], dtype=np.int64)
    w = np.asarray(edge_weight, dtype=np.float32)

    loop = np.arange(N, dtype=np.int64)
    rows = np.concatenate([row, loop])
    cols = np.concatenate([col, loop])
    ws = np.concatenate([w, np.ones(N, np.float32)])
    EE = rows.shape[0]

    # ---- dinv on host (graph-structure preprocessing) ----
    deg = np.zeros(NPAD, np.float32)
    np.add.at(deg, cols, ws)
    deg = deg + (deg == 0)
    dinv = np.sqrt(1.0 / deg).astype(np.float32)
    # partition-major [128, 392]: dinvP[p, nb] = dinv[nb*128+p]
    dinvP = np.ascontiguousarray(dinv.reshape(NPAD // P, P).T)

    # ---- edge streams per (block, half), sorted by src within bucket ----
    blk = cols // P                      # 0..391
    half = (rows >= HALF).astype(np.int64)
    key = blk * 2 + half
    cnt = np.bincount(key, minlength=(NPAD // P) * 2)
    TH = int(-(-cnt.max() // P))         # tiles per half
    CAP = TH * P
    NB = NPAD // P                       # 392 blocks

    src_a = np.full((NB, 2, CAP), DUMMY_SRC % HALF, np.int16)
    col_a = np.zeros((NB, 2, CAP), np.float32)
    w_a = np.zeros((NB, 2, CAP), np.float32)

    # sort by (bucket, src) so gathered rows are ascending within a bucket
    order2 = np.lexsort((rows, key))
    cs2 = np.zeros(NB * 2 + 1, np.int64)
    np.cumsum(cnt, out=cs2[1:])
    pos = np.arange(EE) - cs2[key[order2]]
    kb = key[order2] // 2
    kh = key[order2] % 2
    src_sorted = rows[order2]
    src_rel = np.where(kh == 1, src_sorted - HALF, src_sorted).astype(np.int16)
    src_a[kb, kh, pos] = src_rel
    col_a[kb, kh, pos] = (cols[order2] - kb * P).astype(np.float32)
    w_a[kb, kh, pos] = ws[order2]

    # wrapped int16 index layout for dma_gather: index i -> partition i%16,
    # col i//16, replicated across the 8 groups of 16 partitions.
    IW = CAP // 16
    idx_w = src_a.reshape(NB, 2, IW, 16).transpose(0, 1, 3, 2)  # [NB,2,16,IW]
    idx_w = np.ascontiguousarray(np.tile(idx_w, (1, 1, 8, 1)))  # [NB,2,128,IW]

    # col/w in per-tile scalar layout: [.., 128, 2*TH] where slot (h*TH+t)
    # on partition p = edge t*128+p of half h.  Stored bf16.
    colP = col_a.reshape(NB, 2, TH, P).transpose(3, 0, 1, 2).reshape(P, NB * 2 * TH)
    wfP = w_a.reshape(NB, 2, TH, P).transpose(3, 0, 1, 2).reshape(P, NB * 2 * TH)
    colP = np.ascontiguousarray(colP)
    wfP = np.ascontiguousarray(wfP)

    return dict(TH=TH, CAP=CAP, dinvP=dinvP, idx_w=idx_w, colP=colP, wfP=wfP)


def _host_golden(x, W1, b1, W2, b2, pp, out_dtype=np.float32, quant=True):
    """Numpy re-implementation of the device algorithm (same tiling, same
    bf16 quantization points). For validating the scheme off-device."""
    bf = (lambda a: a.astype(_BF16).astype(np.float32)) if quant else (lambda a: a)
    TH, CAP = pp["TH"], pp["CAP"]
    NB = NPAD // P

    dinv = pp["dinvP"].T.reshape(-1)

    xp = np.zeros((NPAD, FIN), np.float32)
    xp[:N] = x
    h1 = bf(xp) @ bf(W1)                     # bf16 inputs, f32 accum
    hs1 = bf(h1 * dinv[:, None])             # stored bf16

    idx_w = pp["idx_w"]
    colP = pp["colP"]
    wfP = np.asarray(pp["wfP"].astype(_BF16), dtype=np.float32)

    def agg(hs, F):
        out = np.zeros((NPAD, F), np.float32)
        for nb in range(NB):
            acc = np.zeros((P, F), np.float32)
            for hh in range(2):
                iw = idx_w[nb, hh, :16, :]                      # [16, IW]
                flat = iw.T.reshape(-1)[:CAP].astype(np.int64)  # unwrap
                base = 0 if hh == 0 else HALF
                for t in range(TH):
                    oh = np.zeros((P, P), np.float32)
                    c = colP[:, (nb * 2 + hh) * TH + t]
                    wv = wfP[:, (nb * 2 + hh) * TH + t]
                    oh[np.arange(P), c.astype(np.int64)] = wv
                    idxs = flat[t * P:(t + 1) * P]
                    msgs = np.where(idxs[:, None] >= 0, hs[base + np.maximum(idxs, 0)], 0.0)
                    acc += oh.T @ msgs
            out[nb * P:(nb + 1) * P] = acc
        return out

    out1 = agg(hs1, H) * dinv[:, None] + b1[None, :]
    out1 = np.maximum(out1, 0.0)

    h2 = bf(out1) @ bf(W2)
    hs2 = bf(h2 * dinv[:, None])
    out2 = agg(hs2, F2) * dinv[:, None] + b2[None, :]
    return out2[:N].astype(out_dtype)


# ---------------------------------------------------------------------------
# Bass device kernel
# ---------------------------------------------------------------------------

_NC_CACHE = {}

MQ = 1  # SWDGE queues to rotate over


def _build_nc(TH):
    import concourse.bass as bass  # noqa: F401
    import concourse.mybir as mybir
    import concourse.tile as tile
    from concourse import bacc
    from concourse.library_config import mlp

    DT = mybir.dt.bfloat16
    F32 = mybir.dt.float32
    I16 = mybir.dt.int16
    AL = mybir.AluOpType
    AF = mybir.ActivationFunctionType

    CAP = TH * P
    IW = CAP // 16
    NB = NPAD // P           # 392

    nc = bacc.Bacc("TRN2", target_bir_lowering=False, debug=True,
                   num_devices=NCORES)
    xt3_d = nc.dram_tensor("xt3", [2, P, NPAD], DT, kind="ExternalInput")
    w1_d = nc.dram_tensor("w1c", [2, P, H], DT, kind="ExternalInput")
    w2_d = nc.dram_tensor("w2c", [2, P, F2], DT, kind="ExternalInput")
    b1_d = nc.dram_tensor("b1f", [P, H], F32, kind="ExternalInput")
    b2_d = nc.dram_tensor("b2f", [P, F2], F32, kind="ExternalInput")
    iota_d = nc.dram_tensor("iota", [P, P], DT, kind="ExternalInput")
    dinv_d = nc.dram_tensor("dinvP", [P, NB], F32, kind="ExternalInput")
    dinvl_d = nc.dram_tensor("dinvlP", [P, BPC], F32, kind="ExternalInput")
    idx_d = nc.dram_tensor("idxP", [P, BPC * 2 * IW], I16, kind="ExternalInput")
    col_d = nc.dram_tensor("colP", [P, BPC * 2 * TH], F32, kind="ExternalInput")
    wf_d = nc.dram_tensor("wfP", [P, BPC * 2 * TH], F32, kind="ExternalInput")
    out_d = nc.dram_tensor("out2", [SHARD, F2], F32, kind="ExternalOutput")

    qctr = [0]

    def next_q():
        q = qctr[0] % MQ
        qctr[0] += 1
        return q

    with tile.TileContext(nc) as tc:
        with (
            tc.tile_pool(name="dram", bufs=1, space="DRAM") as dpool,
            tc.tile_pool(name="const", bufs=1) as cpool,
            tc.tile_pool(name="xs", bufs=4) as xpool,
            tc.tile_pool(name="hst", bufs=4) as hpool,
            tc.tile_pool(name="msg", bufs=3) as mpool,
            tc.tile_pool(name="oh", bufs=8) as ohpool,
            tc.tile_pool(name="post", bufs=3) as tpool,
            tc.tile_pool(name="ph1", bufs=2, space="PSUM") as ph1p,
            tc.tile_pool(name="pagg", bufs=3, space="PSUM") as paggp,
            tc.tile_pool(name="pc", bufs=3, space="PSUM") as pcp,
        ):
            hs1_tab = dpool.tile([NPAD, H], DT)
            h2in_dram = dpool.tile([SHARD, H], DT)
            hs2_shard = dpool.tile([SHARD, F2], DT)
            hs2_full = dpool.tile([NPAD, F2], DT, addr_space="Shared")

            nc.gpsimd.load_library(mlp)

            # ---- constants ----
            w1_sb = cpool.tile([P, 2 * H], DT)
            nc.sync.dma_start(out=w1_sb[:, 0:H], in_=w1_d[0])
            nc.sync.dma_start(out=w1_sb[:, H:2 * H], in_=w1_d[1])
            w2_sb = cpool.tile([P, 2 * F2], DT)
            nc.sync.dma_start(out=w2_sb[:, 0:F2], in_=w2_d[0])
            nc.sync.dma_start(out=w2_sb[:, F2:2 * F2], in_=w2_d[1])
            b1_sb = cpool.tile([P, H], F32)
            nc.sync.dma_start(out=b1_sb[:], in_=b1_d[:])
            b2_sb = cpool.tile([P, F2], F32)
            nc.sync.dma_start(out=b2_sb[:], in_=b2_d[:])
            iota_sb = cpool.tile([P, P], DT)
            nc.sync.dma_start(out=iota_sb[:], in_=iota_d[:])
            idx_sb = cpool.tile([P, BPC * 2 * IW], I16)
            nc.sync.dma_start(out=idx_sb[:], in_=idx_d[:])
            col_sb = cpool.tile([P, BPC * 2 * TH], F32)
            nc.sync.dma_start(out=col_sb[:], in_=col_d[:])
            wf_sb = cpool.tile([P, BPC * 2 * TH], F32)
            nc.sync.dma_start(out=wf_sb[:], in_=wf_d[:])
            dinv_sb = cpool.tile([P, NB], F32)
            nc.sync.dma_start(out=dinv_sb[:], in_=dinv_d[:])
            dinvl_sb = cpool.tile([P, BPC], F32)
            nc.sync.dma_start(out=dinvl_sb[:], in_=dinvl_d[:])

            # zero the msgs ring once so truncated-gather tail slots are
            # finite (their one-hot weights are 0; 0*finite==0 in the PE)
            for _ in range(3):
                for hh in range(2):
                    mz = mpool.tile([P, TH, H], DT, tag=f"msg{hh}")
                    nc.vector.memset(mz[:], 0)

            # ---- phase A: h1 = x @ W1 (all nodes), hs1 = h1 * dinv ----
            for s in range(NPAD // 512):
                xa = xpool.tile([P, 512], DT, tag="xa")
                xb = xpool.tile([P, 512], DT, tag="xb")
                nc.sync.dma_start(out=xa[:], in_=xt3_d[0][:, s * 512:(s + 1) * 512])
                nc.sync.dma_start(out=xb[:], in_=xt3_d[1][:, s * 512:(s + 1) * 512])
                for q in range(4):
                    nb = s * 4 + q
                    ph = ph1p.tile([P, H], F32)
                    nc.tensor.matmul(ph[:], lhsT=xa[:, q * P:(q + 1) * P],
                                     rhs=w1_sb[:, 0:H], start=True, stop=False)
                    nc.tensor.matmul(ph[:], lhsT=xb[:, q * P:(q + 1) * P],
                                     rhs=w1_sb[:, H:2 * H], start=False, stop=True)
                    hst = hpool.tile([P, H], DT, tag="hst")
                    nc.scalar.activation(hst[:], ph[:], AF.Copy,
                                         scale=dinv_sb[:, nb:nb + 1])
                    nc.sync.dma_start(out=hs1_tab[nb * P:(nb + 1) * P, :], in_=hst[:])

            # ---- phase B: layer-1 aggregation per dest block, with the
            #      layer-2 projection (relu_out @ W2) interleaved ----
            for b in range(BPC):
                msgs = []
                for hh in range(2):
                    m = mpool.tile([P, TH, H], DT, tag=f"msg{hh}")
                    src = hs1_tab[0:HALF, :] if hh == 0 else hs1_tab[HALF:NPAD, :]
                    nc.gpsimd.dma_gather(
                        m[:], src, idx_sb[:, (b * 2 + hh) * IW:(b * 2 + hh + 1) * IW],
                        CAP, CAP, H, single_packet=False, queue_num=next_q())
                    msgs.append(m)
                pagg = paggp.tile([P, H], F32)
                for t in range(2 * TH):
                    hh, tt = (0, t) if t < TH else (1, t - TH)
                    oh = ohpool.tile([P, P], DT, tag="oh")
                    sc = (b * 2 + hh) * TH + tt
                    nc.vector.tensor_scalar(oh[:], iota_sb[:], col_sb[:, sc:sc + 1],
                                            wf_sb[:, sc:sc + 1], AL.is_equal, AL.mult)
                    nc.tensor.matmul(pagg[:], lhsT=oh[:], rhs=msgs[hh][:, tt, :],
                                     start=(t == 0), stop=(t == 2 * TH - 1))
                t1 = tpool.tile([P, H], F32, tag="t1")
                nc.vector.tensor_scalar(t1[:], pagg[:], dinvl_sb[:, b:b + 1], None,
                                        AL.mult)
                t2 = tpool.tile([P, H], F32, tag="t2")
                nc.vector.tensor_tensor(t2[:], t1[:], b1_sb[:], AL.add)
                rl = hpool.tile([P, H], DT, tag="rl")
                nc.scalar.activation(rl[:], t2[:], AF.Relu)
                nc.sync.dma_start(out=h2in_dram[b * P:(b + 1) * P, :], in_=rl[:])

                # layer-2 projection for this block
                ph2 = pcp.tile([P, F2], F32, tag="pc")
                for c2 in range(2):
                    at = ohpool.tile([P, P], DT, tag="at")
                    nc.sync.dma_start(
                        out=at[:],
                        in_=h2in_dram[b * P:(b + 1) * P, c2 * P:(c2 + 1) * P],
                        transpose=True)
                    nc.tensor.matmul(ph2[:], lhsT=at[:],
                                     rhs=w2_sb[:, c2 * F2:(c2 + 1) * F2],
                                     start=(c2 == 0), stop=(c2 == 1))
                hsb = hpool.tile([P, F2], DT, tag="hsb")
                nc.scalar.activation(hsb[:], ph2[:], AF.Copy,
                                     scale=dinvl_sb[:, b:b + 1])
                nc.sync.dma_start(out=hs2_shard[b * P:(b + 1) * P, :], in_=hsb[:])

            # ---- phase D: exchange hs2 shards ----
            nc.gpsimd.collective_compute(
                "AllGather", AL.bypass,
                replica_groups=[list(range(NCORES))],
                ins=[hs2_shard[:]],
                outs=[hs2_full[:]],
            )

            # ---- phase E: layer-2 aggregation per dest block ----
            for b in range(BPC):
                msgs = []
                for hh in range(2):
                    m = mpool.tile([P, TH, F2], DT, tag=f"msg{hh}")
                    src = hs2_full[0:HALF, :] if hh == 0 else hs2_full[HALF:NPAD, :]
                    nc.gpsimd.dma_gather(
                        m[:], src, idx_sb[:, (b * 2 + hh) * IW:(b * 2 + hh + 1) * IW],
                        CAP, CAP, F2, single_packet=False, queue_num=next_q())
                    msgs.append(m)
                pagg2 = pcp.tile([P, F2], F32, tag="pc")
                for t in range(2 * TH):
                    hh, tt = (0, t) if t < TH else (1, t - TH)
                    oh = ohpool.tile([P, P], DT, tag="oh")
                    sc = (b * 2 + hh) * TH + tt
                    nc.vector.tensor_scalar(oh[:], iota_sb[:], col_sb[:, sc:sc + 1],
                                            wf_sb[:, sc:sc + 1], AL.is_equal, AL.mult)
                    nc.tensor.matmul(pagg2[:], lhsT=oh[:], rhs=msgs[hh][:, tt, :],
                                     start=(t == 0), stop=(t == 2 * TH - 1))
                o1 = tpool.tile([P, F2], F32, tag="o1")
                nc.vector.tensor_scalar(o1[:], pagg2[:], dinvl_sb[:, b:b + 1], None,
                                        AL.mult)
                o2 = tpool.tile([P, F2], F32, tag="o2")
                nc.vector.tensor_tensor(o2[:], o1[:], b2_sb[:], AL.add)
                nc.sync.dma_start(out=out_d[b * P:(b + 1) * P, :], in_=o2[:])

    nc.compile()
    return nc


def _make_inputs(x, W1, b1, W2, b2, pp):
    """Per-core input maps."""
    TH = pp["TH"]
    IW = (TH * P) // 16
    NB = NPAD // P

    xp = np.zeros((NPAD, FIN), np.float32)
    xp[:N] = x
    xt3 = np.ascontiguousarray(
        xp.T.reshape(2, P, NPAD).astype(_BF16))
    w1c = np.ascontiguousarray(W1.reshape(2, P, H).astype(_BF16))
    w2c = np.ascontiguousarray(W2.reshape(2, P, F2).astype(_BF16))
    b1f = np.ascontiguousarray(np.tile(b1[None, :], (P, 1)).astype(np.float32))
    b2f = np.ascontiguousarray(np.tile(b2[None, :], (P, 1)).astype(np.float32))
    iota = np.tile(np.arange(P, dtype=np.float32)[None, :], (P, 1)).astype(_BF16)

    dinvP = pp["dinvP"]
    idx_w = pp["idx_w"]        # [NB, 2, 128, IW]
    colP = pp["colP"]          # [128, NB*2*TH] bf16
    wfP = pp["wfP"]

    in_maps = []
    for c in range(NCORES):
        b0 = c * BPC
        idxP = np.ascontiguousarray(
            idx_w[b0:b0 + BPC].transpose(2, 0, 1, 3).reshape(P, BPC * 2 * IW))
        in_maps.append({
            "xt3": xt3,
            "w1c": w1c,
            "w2c": w2c,
            "b1f": b1f,
            "b2f": b2f,
            "iota": iota,
            "dinvP": dinvP,
            "dinvlP": np.ascontiguousarray(dinvP[:, b0:b0 + BPC]),
            "idxP": idxP,
            "colP": np.ascontiguousarray(colP[:, b0 * 2 * TH:(b0 + BPC) * 2 * TH]),
            "wfP": np.ascontiguousarray(wfP[:, b0 * 2 * TH:(b0 + BPC) * 2 * TH]),
        })
    return in_maps


def kernel(x, edge_index, edge_weight, W1, b1, W2, b2, _trace=False):
    from concourse.bass_utils import run_bass_kernel_spmd

    x = np.asarray(x, dtype=np.float32)
    W1 = np.asarray(W1, dtype=np.float32)
    b1 = np.asarray(b1, dtype=np.float32)
    W2 = np.asarray(W2, dtype=np.float32)
    b2 = np.asarray(b2, dtype=np.float32)

    pp = _preprocess(np.asarray(edge_index), np.asarray(edge_weight))
    key = pp["TH"]
    if key not in _NC_CACHE:
        _NC_CACHE[key] = _build_nc(key)
    nc = _NC_CACHE[key]

    in_maps = _make_inputs(x, W1, b1, W2, b2, pp)
    res = run_bass_kernel_spmd(nc, in_maps, list(range(NCORES)), trace=_trace)
    out = np.concatenate([res.results[c]["out2"] for c in range(NCORES)], axis=0)
    if _trace:
        kernel._last_result = res
    return np.ascontiguousarray(out[:N])


# revision 9
# speedup vs baseline: 1.2279x; 1.1808x over previous
"""GCN encoder (2-layer) Bass kernel for Trainium2, 8 NeuronCores.

Strategy (graph/data parallel, per sharding hint):
  - Nodes padded to NPAD=50176 and sharded by contiguous range: core c owns
    destination nodes [c*6272, (c+1)*6272) = 49 blocks of 128.
  - Edges (incl. self-loops) are bucketed by destination block and by source
    half (dma_gather indices are int16, so the feature table is gathered in
    two halves of 25088 rows each), sorted by source within each bucket for
    HBM page locality.  Edges with src==dst ("diagonal" edges, incl. all
    self-loops) are pulled out of the buckets and handled by one per-block
    diagonal matmul instead (no gather needed).  Each (block,half) bucket is
    padded to V = max-over-cores ceil128(count) so all 8 cores run one
    identical SPMD program with per-gather-exact descriptor counts (the Q7
    descriptor-generation on the GpSimd engine is the kernel bottleneck).
  - Per layer: h = x @ W (dense matmul, PSUM f32), table hs = h * dinv[src]
    stored in HBM (bf16); per destination block, edge messages are fetched
    with dma_gather (128 edges/tile, edge-major) and segment-summed on the
    TensorEngine via one-hot matmuls: onehot[k,d] = w[k] * (col[k]==d), so
    PSUM[d,f] += sum_k w[k]*hs[src_k][f]. Post: * dinv[dest] + bias (+relu).
  - The layer-2 projection (relu_out @ W2) is interleaved into the layer-1
    block loop; hs2 shards are then exchanged with an AllGather collective.
  - dinv = rsqrt(deg) is precomputed on the host (graph-structure preproc).

kernel(**inputs) takes the FULL inputs and returns the FULL [50000,128] f32
output; all sharding/gather happens inside.
"""

import sys

sys.path.insert(0, "/opt/trn_rl_repo")

import numpy as np
import ml_dtypes

P = 128
NCORES = 8
BPC = 49                 # dest blocks per core
SHARD = BPC * P          # 6272
NPAD = NCORES * SHARD    # 50176
HALF = NPAD // 2         # 25088
N = 50000
FIN = 256
H = 256                  # layer-1 output width
F2 = 128                 # layer-2 output width
DUMMY_SRC = N + 8        # a zero (pad) node, used as src for pad edges

_BF16 = ml_dtypes.bfloat16


def _preprocess(edge_index, edge_weight):
    """Build all per-core device input arrays from the edge list."""
    row = np.asarray(edge_index[0], dtype=np.int64)
    col = np.asarray(edge_index[1], dtype=np.int64)
    w = np.asarray(edge_weight, dtype=np.float32)

    loop = np.arange(N, dtype=np.int64)
    rows = np.concatenate([row, loop])
    cols = np.concatenate([col, loop])
    ws = np.concatenate([w, np.ones(N, np.float32)])

    # ---- dinv on host (graph-structure preprocessing) ----
    deg = np.zeros(NPAD, np.float32)
    np.add.at(deg, cols, ws)
    deg = deg + (deg == 0)
    dinv = np.sqrt(1.0 / deg).astype(np.float32)
    # partition-major [128, 392]: dinvP[p, nb] = dinv[nb*128+p]
    dinvP = np.ascontiguousarray(dinv.reshape(NPAD // P, P).T)

    # ---- diagonal (src==dst) edges -> per-node diag weight ----
    isdiag = rows == cols
    diagw = np.zeros(NPAD, np.float32)
    np.add.at(diagw, cols[isdiag], ws[isdiag])
    diagwP = np.ascontiguousarray(diagw.reshape(NPAD // P, P).T)

    rows = rows[~isdiag]
    cols = cols[~isdiag]
    ws = ws[~isdiag]
    EE = rows.shape[0]

    # ---- edge streams per (block, half), sorted by src within bucket ----
    blk = cols // P                      # 0..391
    half = (rows >= HALF).astype(np.int64)
    key = blk * 2 + half
    cnt = np.bincount(key, minlength=(NPAD // P) * 2)
    TH = int(-(-cnt.max() // P))         # tiles per half (max over all)
    CAP = TH * P
    NB = NPAD // P                       # 392 blocks

    src_a = np.full((NB, 2, CAP), DUMMY_SRC % HALF, np.int16)
    col_a = np.zeros((NB, 2, CAP), np.float32)
    w_a = np.zeros((NB, 2, CAP), np.float32)

    # sort by (bucket, src) so gathered rows are ascending within a bucket
    order2 = np.lexsort((rows, key))
    cs2 = np.zeros(NB * 2 + 1, np.int64)
    np.cumsum(cnt, out=cs2[1:])
    pos = np.arange(EE) - cs2[key[order2]]
    kb = key[order2] // 2
    kh = key[order2] % 2
    src_sorted = rows[order2]
    src_rel = np.where(kh == 1, src_sorted - HALF, src_sorted).astype(np.int16)
    src_a[kb, kh, pos] = src_rel
    col_a[kb, kh, pos] = (cols[order2] - kb * P).astype(np.float32)
    w_a[kb, kh, pos] = ws[order2]

    # per-(block,half) gather length: max over the 8 cores at the same block
    # position of ceil128(count); identical across cores -> static program.
    cnt2 = cnt.reshape(NCORES, BPC, 2)
    tcnt = -(-cnt2 // P)                       # tiles, [8, 49, 2]
    tmax = tcnt.max(axis=0)                    # [49, 2]
    V = tmax * P                               # gather num_idxs per position

    # wrapped int16 index layout for dma_gather: index i -> partition i%16,
    # col i//16, replicated across the 8 groups of 16 partitions.
    IW = CAP // 16
    idx_w = src_a.reshape(NB, 2, IW, 16).transpose(0, 1, 3, 2)  # [NB,2,16,IW]
    idx_w = np.ascontiguousarray(np.tile(idx_w, (1, 1, 8, 1)))  # [NB,2,128,IW]

    # col/w in per-tile scalar layout: [.., 128, 2*TH] where slot (h*TH+t)
    # on partition p = edge t*128+p of half h.
    colP = col_a.reshape(NB, 2, TH, P).transpose(3, 0, 1, 2).reshape(P, NB * 2 * TH)
    wfP = w_a.reshape(NB, 2, TH, P).transpose(3, 0, 1, 2).reshape(P, NB * 2 * TH)
    colP = np.ascontiguousarray(colP)
    wfP = np.ascontiguousarray(wfP)

    return dict(TH=TH, CAP=CAP, V=V, dinvP=dinvP, diagwP=diagwP,
                idx_w=idx_w, colP=colP, wfP=wfP)


def _host_golden(x, W1, b1, W2, b2, pp, out_dtype=np.float32, quant=True):
    """Numpy re-implementation of the device algorithm (same tiling, same
    bf16 quantization points). For validating the scheme off-device."""
    bf = (lambda a: a.astype(_BF16).astype(np.float32)) if quant else (lambda a: a)
    TH, CAP, V = pp["TH"], pp["CAP"], pp["V"]
    NB = NPAD // P

    dinv = pp["dinvP"].T.reshape(-1)
    diagw = pp["diagwP"].T.reshape(-1)

    xp = np.zeros((NPAD, FIN), np.float32)
    xp[:N] = x
    h1 = bf(xp) @ bf(W1)                     # bf16 inputs, f32 accum
    hs1 = bf(h1 * dinv[:, None])             # stored bf16

    idx_w = pp["idx_w"]
    colP = pp["colP"]
    wfP = np.asarray(pp["wfP"].astype(_BF16), dtype=np.float32)

    def agg(hs, F):
        out = np.zeros((NPAD, F), np.float32)
        for nb in range(NB):
            b = nb % BPC
            acc = (bf(diagw[nb * P:(nb + 1) * P])[:, None]
                   * hs[nb * P:(nb + 1) * P, :F])
            for hh in range(2):
                iw = idx_w[nb, hh, :16, :]                      # [16, IW]
                flat = iw.T.reshape(-1)[:CAP].astype(np.int64)  # unwrap
                base = 0 if hh == 0 else HALF
                for t in range(V[b, hh] // P):
                    oh = np.zeros((P, P), np.float32)
                    c = colP[:, (nb * 2 + hh) * TH + t]
                    wv = wfP[:, (nb * 2 + hh) * TH + t]
                    oh[np.arange(P), c.astype(np.int64)] = wv
                    msgs = hs[base + flat[t * P:(t + 1) * P]]
                    acc = acc + oh.T @ msgs
            out[nb * P:(nb + 1) * P] = acc
        return out

    out1 = agg(hs1, H) * dinv[:, None] + b1[None, :]
    out1 = np.maximum(out1, 0.0)

    h2 = bf(out1) @ bf(W2)
    hs2 = bf(h2 * dinv[:, None])
    out2 = agg(hs2, F2) * dinv[:, None] + b2[None, :]
    return out2[:N].astype(out_dtype)


# ---------------------------------------------------------------------------
# Bass device kernel
# ---------------------------------------------------------------------------

_NC_CACHE = {}


def _build_nc(TH, Vkey):
    import concourse.bass as bass  # noqa: F401
    import concourse.mybir as mybir
    import concourse.tile as tile
    from concourse import bacc
    from concourse.library_config import mlp

    DT = mybir.dt.bfloat16
    F32 = mybir.dt.float32
    I16 = mybir.dt.int16
    AL = mybir.AluOpType
    AF = mybir.ActivationFunctionType

    V = np.array(Vkey, dtype=np.int64).reshape(BPC, 2)
    CAP = TH * P
    IW = CAP // 16
    NB = NPAD // P           # 392

    nc = bacc.Bacc("TRN2", target_bir_lowering=False, debug=True,
                   num_devices=NCORES)
    xt3_d = nc.dram_tensor("xt3", [2, P, NPAD], DT, kind="ExternalInput")
    w1_d = nc.dram_tensor("w1c", [2, P, H], DT, kind="ExternalInput")
    w2_d = nc.dram_tensor("w2c", [2, P, F2], DT, kind="ExternalInput")
    b1_d = nc.dram_tensor("b1f", [P, H], F32, kind="ExternalInput")
    b2_d = nc.dram_tensor("b2f", [P, F2], F32, kind="ExternalInput")
    iota_d = nc.dram_tensor("iota", [P, P], DT, kind="ExternalInput")
    pidx_d = nc.dram_tensor("pidxf", [P, 1], F32, kind="ExternalInput")
    dinv_d = nc.dram_tensor("dinvP", [P, NB], F32, kind="ExternalInput")
    dinvl_d = nc.dram_tensor("dinvlP", [P, BPC], F32, kind="ExternalInput")
    diagwl_d = nc.dram_tensor("diagwlP", [P, BPC], F32, kind="ExternalInput")
    idx_d = nc.dram_tensor("idxP", [P, BPC * 2 * IW], I16, kind="ExternalInput")
    col_d = nc.dram_tensor("colP", [P, BPC * 2 * TH], F32, kind="ExternalInput")
    wf_d = nc.dram_tensor("wfP", [P, BPC * 2 * TH], F32, kind="ExternalInput")
    out_d = nc.dram_tensor("out2", [SHARD, F2], F32, kind="ExternalOutput")

    with tile.TileContext(nc) as tc:
        with (
            tc.tile_pool(name="dram", bufs=1, space="DRAM") as dpool,
            tc.tile_pool(name="const", bufs=1) as cpool,
            tc.tile_pool(name="xs", bufs=4) as xpool,
            tc.tile_pool(name="hst", bufs=4) as hpool,
            tc.tile_pool(name="msg", bufs=3) as mpool,
            tc.tile_pool(name="oh", bufs=8) as ohpool,
            tc.tile_pool(name="post", bufs=3) as tpool,
            tc.tile_pool(name="ph1", bufs=2, space="PSUM") as ph1p,
            tc.tile_pool(name="pagg", bufs=3, space="PSUM") as paggp,
            tc.tile_pool(name="pc", bufs=3, space="PSUM") as pcp,
        ):
            hs1_tab = dpool.tile([NPAD, H], DT)
            h2in_dram = dpool.tile([SHARD, H], DT)
            hs2_shard = dpool.tile([SHARD, F2], DT)
            hs2_full = dpool.tile([NPAD, F2], DT, addr_space="Shared")

            nc.gpsimd.load_library(mlp)

            # ---- constants ----
            w1_sb = cpool.tile([P, 2 * H], DT)
            nc.sync.dma_start(out=w1_sb[:, 0:H], in_=w1_d[0])
            nc.sync.dma_start(out=w1_sb[:, H:2 * H], in_=w1_d[1])
            w2_sb = cpool.tile([P, 2 * F2], DT)
            nc.sync.dma_start(out=w2_sb[:, 0:F2], in_=w2_d[0])
            nc.sync.dma_start(out=w2_sb[:, F2:2 * F2], in_=w2_d[1])
            b1_sb = cpool.tile([P, H], F32)
            nc.sync.dma_start(out=b1_sb[:], in_=b1_d[:])
            b2_sb = cpool.tile([P, F2], F32)
            nc.sync.dma_start(out=b2_sb[:], in_=b2_d[:])
            iota_sb = cpool.tile([P, P], DT)
            nc.sync.dma_start(out=iota_sb[:], in_=iota_d[:])
            pidx_sb = cpool.tile([P, 1], F32)
            nc.sync.dma_start(out=pidx_sb[:], in_=pidx_d[:])
            idx_sb = cpool.tile([P, BPC * 2 * IW], I16)
            nc.sync.dma_start(out=idx_sb[:], in_=idx_d[:])
            col_sb = cpool.tile([P, BPC * 2 * TH], F32)
            nc.sync.dma_start(out=col_sb[:], in_=col_d[:])
            wf_sb = cpool.tile([P, BPC * 2 * TH], F32)
            nc.sync.dma_start(out=wf_sb[:], in_=wf_d[:])
            dinv_sb = cpool.tile([P, NB], F32)
            nc.sync.dma_start(out=dinv_sb[:], in_=dinv_d[:])
            dinvl_sb = cpool.tile([P, BPC], F32)
            nc.sync.dma_start(out=dinvl_sb[:], in_=dinvl_d[:])
            diagwl_sb = cpool.tile([P, BPC], F32)
            nc.sync.dma_start(out=diagwl_sb[:], in_=diagwl_d[:])

            # ---- phase A: h1 = x @ W1 (all nodes), hs1 = h1 * dinv ----
            for s in range(NPAD // 512):
                xa = xpool.tile([P, 512], DT, tag="xa")
                xb = xpool.tile([P, 512], DT, tag="xb")
                nc.sync.dma_start(out=xa[:], in_=xt3_d[0][:, s * 512:(s + 1) * 512])
                nc.sync.dma_start(out=xb[:], in_=xt3_d[1][:, s * 512:(s + 1) * 512])
                for q in range(4):
                    nb = s * 4 + q
                    ph = ph1p.tile([P, H], F32)
                    nc.tensor.matmul(ph[:], lhsT=xa[:, q * P:(q + 1) * P],
                                     rhs=w1_sb[:, 0:H], start=True, stop=False)
                    nc.tensor.matmul(ph[:], lhsT=xb[:, q * P:(q + 1) * P],
                                     rhs=w1_sb[:, H:2 * H], start=False, stop=True)
                    hst = hpool.tile([P, H], DT, tag="hst")
                    if nb % 2 == 0:
                        nc.scalar.activation(hst[:], ph[:], AF.Copy,
                                             scale=dinv_sb[:, nb:nb + 1])
                    else:
                        nc.vector.tensor_scalar(hst[:], ph[:],
                                                dinv_sb[:, nb:nb + 1], None,
                                                AL.mult)
                    nc.sync.dma_start(out=hs1_tab[nb * P:(nb + 1) * P, :], in_=hst[:])

            pid = nc.sync.partition_id()

            def aggregate(b, tab, tab_own, F, psum_pool, psum_tag):
                """Edge aggregation for dest block b from table `tab` (+ the
                diagonal term from the core's own block). Returns PSUM tile."""
                msgs = []
                for hh in range(2):
                    nt = int(V[b, hh]) // P
                    m = mpool.tile([P, TH, F], DT, tag=f"msg{hh}")
                    src = tab[0:HALF, :] if hh == 0 else tab[HALF:NPAD, :]
                    nc.gpsimd.dma_gather(
                        m[:, 0:nt, :], src,
                        idx_sb[:, (b * 2 + hh) * IW:(b * 2 + hh) * IW + nt * 8],
                        int(V[b, hh]), int(V[b, hh]), F, single_packet=False)
                    msgs.append(m)
                own = ohpool.tile([P, F], DT, tag="own")
                if tab_own is None:
                    nc.sync.dma_start(
                        out=own[:],
                        in_=tab[bass.ds((pid * BPC + b) * P, P), 0:F])
                else:
                    nc.sync.dma_start(
                        out=own[:], in_=tab_own[b * P:(b + 1) * P, 0:F])
                dg = ohpool.tile([P, P], DT, tag="oh")
                nc.vector.tensor_scalar(dg[:], iota_sb[:], pidx_sb[:, 0:1],
                                        diagwl_sb[:, b:b + 1], AL.is_equal,
                                        AL.mult)
                pagg = psum_pool.tile([P, F], F32, tag=psum_tag)
                nc.tensor.matmul(pagg[:], lhsT=dg[:], rhs=own[:],
                                 start=True, stop=False)
                ntot = (int(V[b, 0]) + int(V[b, 1])) // P
                t = 0
                for hh in range(2):
                    for tt in range(int(V[b, hh]) // P):
                        oh = ohpool.tile([P, P], DT, tag="oh")
                        sc = (b * 2 + hh) * TH + tt
                        nc.vector.tensor_scalar(oh[:], iota_sb[:],
                                                col_sb[:, sc:sc + 1],
                                                wf_sb[:, sc:sc + 1],
                                                AL.is_equal, AL.mult)
                        nc.tensor.matmul(pagg[:], lhsT=oh[:],
                                         rhs=msgs[hh][:, tt, :],
                                         start=False, stop=(t == ntot - 1))
                        t += 1
                return pagg

            # ---- phase B: layer-1 aggregation per dest block, with the
            #      layer-2 projection (relu_out @ W2) interleaved ----
            for b in range(BPC):
                pagg = aggregate(b, hs1_tab, None, H, paggp, "")
                t1 = tpool.tile([P, H], F32, tag="t1")
                nc.vector.tensor_scalar(t1[:], pagg[:], dinvl_sb[:, b:b + 1], None,
                                        AL.mult)
                t2 = tpool.tile([P, H], F32, tag="t2")
                nc.vector.tensor_tensor(t2[:], t1[:], b1_sb[:], AL.add)
                rl = hpool.tile([P, H], DT, tag="rl")
                nc.scalar.activation(rl[:], t2[:], AF.Relu)
                nc.sync.dma_start(out=h2in_dram[b * P:(b + 1) * P, :], in_=rl[:])

                # layer-2 projection for this block
                ph2 = pcp.tile([P, F2], F32, tag="pc")
                for c2 in range(2):
                    at = ohpool.tile([P, P], DT, tag="at")
                    nc.sync.dma_start(
                        out=at[:],
                        in_=h2in_dram[b * P:(b + 1) * P, c2 * P:(c2 + 1) * P],
                        transpose=True)
                    nc.tensor.matmul(ph2[:], lhsT=at[:],
                                     rhs=w2_sb[:, c2 * F2:(c2 + 1) * F2],
                                     start=(c2 == 0), stop=(c2 == 1))
                hsb = hpool.tile([P, F2], DT, tag="hsb")
                nc.scalar.activation(hsb[:], ph2[:], AF.Copy,
                                     scale=dinvl_sb[:, b:b + 1])
                nc.sync.dma_start(out=hs2_shard[b * P:(b + 1) * P, :], in_=hsb[:])

            # ---- phase D: exchange hs2 shards ----
            nc.gpsimd.collective_compute(
                "AllGather", AL.bypass,
                replica_groups=[list(range(NCORES))],
                ins=[hs2_shard[:]],
                outs=[hs2_full[:]],
            )

            # ---- phase E: layer-2 aggregation per dest block ----
            for b in range(BPC):
                pagg2 = aggregate(b, hs2_full, hs2_shard, F2, pcp, "pc")
                o1 = tpool.tile([P, F2], F32, tag="o1")
                nc.vector.tensor_scalar(o1[:], pagg2[:], dinvl_sb[:, b:b + 1], None,
                                        AL.mult)
                o2 = tpool.tile([P, F2], F32, tag="o2")
                nc.vector.tensor_tensor(o2[:], o1[:], b2_sb[:], AL.add)
                nc.sync.dma_start(out=out_d[b * P:(b + 1) * P, :], in_=o2[:])

    nc.compile()
    return nc


def _make_inputs(x, W1, b1, W2, b2, pp):
    """Per-core input maps."""
    TH = pp["TH"]
    IW = (TH * P) // 16
    NB = NPAD // P

    xp = np.zeros((NPAD, FIN), np.float32)
    xp[:N] = x
    xt3 = np.ascontiguousarray(
        xp.T.reshape(2, P, NPAD).astype(_BF16))
    w1c = np.ascontiguousarray(W1.reshape(2, P, H).astype(_BF16))
    w2c = np.ascontiguousarray(W2.reshape(2, P, F2).astype(_BF16))
    b1f = np.ascontiguousarray(np.tile(b1[None, :], (P, 1)).astype(np.float32))
    b2f = np.ascontiguousarray(np.tile(b2[None, :], (P, 1)).astype(np.float32))
    iota = np.tile(np.arange(P, dtype=np.float32)[None, :], (P, 1)).astype(_BF16)
    pidxf = np.arange(P, dtype=np.float32)[:, None].copy()

    dinvP = pp["dinvP"]
    diagwP = pp["diagwP"]
    idx_w = pp["idx_w"]        # [NB, 2, 128, IW]
    colP = pp["colP"]          # [128, NB*2*TH]
    wfP = pp["wfP"]

    in_maps = []
    for c in range(NCORES):
        b0 = c * BPC
        idxP = np.ascontiguousarray(
            idx_w[b0:b0 + BPC].transpose(2, 0, 1, 3).reshape(P, BPC * 2 * IW))
        in_maps.append({
            "xt3": xt3,
            "w1c": w1c,
            "w2c": w2c,
            "b1f": b1f,
            "b2f": b2f,
            "iota": iota,
            "pidxf": pidxf,
            "dinvP": dinvP,
            "dinvlP": np.ascontiguousarray(dinvP[:, b0:b0 + BPC]),
            "diagwlP": np.ascontiguousarray(diagwP[:, b0:b0 + BPC]),
            "idxP": idxP,
            "colP": np.ascontiguousarray(colP[:, b0 * 2 * TH:(b0 + BPC) * 2 * TH]),
            "wfP": np.ascontiguousarray(wfP[:, b0 * 2 * TH:(b0 + BPC) * 2 * TH]),
        })
    return in_maps


def kernel(x, edge_index, edge_weight, W1, b1, W2, b2, _trace=False):
    from concourse.bass_utils import run_bass_kernel_spmd

    x = np.asarray(x, dtype=np.float32)
    W1 = np.asarray(W1, dtype=np.float32)
    b1 = np.asarray(b1, dtype=np.float32)
    W2 = np.asarray(W2, dtype=np.float32)
    b2 = np.asarray(b2, dtype=np.float32)

    pp = _preprocess(np.asarray(edge_index), np.asarray(edge_weight))
    key = (pp["TH"], tuple(pp["V"].reshape(-1).tolist()))
    if key not in _NC_CACHE:
        _NC_CACHE[key] = _build_nc(*key)
    nc = _NC_CACHE[key]

    in_maps = _make_inputs(x, W1, b1, W2, b2, pp)
    res = run_bass_kernel_spmd(nc, in_maps, list(range(NCORES)), trace=_trace)
    out = np.concatenate([res.results[c]["out2"] for c in range(NCORES)], axis=0)
    if _trace:
        kernel._last_result = res
    return np.ascontiguousarray(out[:N])


# revision 10
# speedup vs baseline: 1.2599x; 1.0261x over previous
"""GCN encoder (2-layer) Bass kernel for Trainium2, 8 NeuronCores.

Strategy (graph/data parallel, per sharding hint):
  - Nodes padded to NPAD=50176 and sharded by contiguous range: core c owns
    destination nodes [c*6272, (c+1)*6272) = 49 blocks of 128.
  - Edges (incl. self-loops) are bucketed by destination block and by source
    half (dma_gather indices are int16, so the feature table is gathered in
    two halves of 25088 rows each), sorted by source within each bucket for
    HBM page locality.  Edges with src==dst ("diagonal" edges, incl. all
    self-loops) are pulled out of the buckets and handled by one per-block
    diagonal matmul instead (no gather needed).  Each (block,half) bucket is
    padded to V = max-over-cores ceil128(count) so all 8 cores run one
    identical SPMD program with per-gather-exact descriptor counts (the Q7
    descriptor-generation on the GpSimd engine is the kernel bottleneck).
  - Per layer: h = x @ W (dense matmul, PSUM f32), table hs = h * dinv[src]
    stored in HBM (bf16); per destination block, edge messages are fetched
    with dma_gather (128 edges/tile, edge-major) and segment-summed on the
    TensorEngine via one-hot matmuls: onehot[k,d] = w[k] * (col[k]==d), so
    PSUM[d,f] += sum_k w[k]*hs[src_k][f]. Post: * dinv[dest] + bias (+relu).
  - The layer-2 projection (relu_out @ W2) is interleaved into the layer-1
    block loop; hs2 shards are then exchanged with an AllGather collective.
  - dinv = rsqrt(deg) is precomputed on the host (graph-structure preproc).

kernel(**inputs) takes the FULL inputs and returns the FULL [50000,128] f32
output; all sharding/gather happens inside.
"""

import sys

sys.path.insert(0, "/opt/trn_rl_repo")

import numpy as np
import ml_dtypes

P = 128
NCORES = 8
BPC = 49                 # dest blocks per core
SHARD = BPC * P          # 6272
NPAD = NCORES * SHARD    # 50176
HALF = NPAD // 2         # 25088
N = 50000
FIN = 256
H = 256                  # layer-1 output width
F2 = 128                 # layer-2 output width
DUMMY_SRC = N + 8        # a zero (pad) node, used as src for pad edges

_BF16 = ml_dtypes.bfloat16


def _preprocess(edge_index, edge_weight):
    """Build all per-core device input arrays from the edge list."""
    row = np.asarray(edge_index[0], dtype=np.int64)
    col = np.asarray(edge_index[1], dtype=np.int64)
    w = np.asarray(edge_weight, dtype=np.float32)

    loop = np.arange(N, dtype=np.int64)
    rows = np.concatenate([row, loop])
    cols = np.concatenate([col, loop])
    ws = np.concatenate([w, np.ones(N, np.float32)])

    # ---- dinv on host (graph-structure preprocessing) ----
    deg = np.zeros(NPAD, np.float32)
    np.add.at(deg, cols, ws)
    deg = deg + (deg == 0)
    dinv = np.sqrt(1.0 / deg).astype(np.float32)
    # partition-major [128, 392]: dinvP[p, nb] = dinv[nb*128+p]
    dinvP = np.ascontiguousarray(dinv.reshape(NPAD // P, P).T)

    # ---- diagonal (src==dst) edges -> per-node diag weight ----
    isdiag = rows == cols
    diagw = np.zeros(NPAD, np.float32)
    np.add.at(diagw, cols[isdiag], ws[isdiag])
    diagwP = np.ascontiguousarray(diagw.reshape(NPAD // P, P).T)

    rows = rows[~isdiag]
    cols = cols[~isdiag]
    ws = ws[~isdiag]
    EE = rows.shape[0]

    # ---- edge streams per (block, half), sorted by src within bucket ----
    blk = cols // P                      # 0..391
    half = (rows >= HALF).astype(np.int64)
    key = blk * 2 + half
    cnt = np.bincount(key, minlength=(NPAD // P) * 2)
    TH = int(-(-cnt.max() // P))         # tiles per half (max over all)
    CAP = TH * P
    NB = NPAD // P                       # 392 blocks

    src_a = np.full((NB, 2, CAP), DUMMY_SRC % HALF, np.int16)
    col_a = np.zeros((NB, 2, CAP), np.float32)
    w_a = np.zeros((NB, 2, CAP), np.float32)

    # sort by (bucket, src) so gathered rows are ascending within a bucket
    order2 = np.lexsort((rows, key))
    cs2 = np.zeros(NB * 2 + 1, np.int64)
    np.cumsum(cnt, out=cs2[1:])
    pos = np.arange(EE) - cs2[key[order2]]
    kb = key[order2] // 2
    kh = key[order2] % 2
    src_sorted = rows[order2]
    src_rel = np.where(kh == 1, src_sorted - HALF, src_sorted).astype(np.int16)
    src_a[kb, kh, pos] = src_rel
    col_a[kb, kh, pos] = (cols[order2] - kb * P).astype(np.float32)
    w_a[kb, kh, pos] = ws[order2]

    # per-(block,half) gather length: max over the 8 cores at the same block
    # position of ceil128(count); identical across cores -> static program.
    cnt2 = cnt.reshape(NCORES, BPC, 2)
    tcnt = -(-cnt2 // P)                       # tiles, [8, 49, 2]
    tmax = tcnt.max(axis=0)                    # [49, 2]
    V = tmax * P                               # gather num_idxs per position

    # wrapped int16 index layout for dma_gather: index i -> partition i%16,
    # col i//16, replicated across the 8 groups of 16 partitions.
    IW = CAP // 16
    idx_w = src_a.reshape(NB, 2, IW, 16).transpose(0, 1, 3, 2)  # [NB,2,16,IW]
    idx_w = np.ascontiguousarray(np.tile(idx_w, (1, 1, 8, 1)))  # [NB,2,128,IW]

    # col/w in per-tile scalar layout: [.., 128, 2*TH] where slot (h*TH+t)
    # on partition p = edge t*128+p of half h.
    colP = col_a.reshape(NB, 2, TH, P).transpose(3, 0, 1, 2).reshape(P, NB * 2 * TH)
    wfP = w_a.reshape(NB, 2, TH, P).transpose(3, 0, 1, 2).reshape(P, NB * 2 * TH)
    colP = np.ascontiguousarray(colP)
    wfP = np.ascontiguousarray(wfP)

    return dict(TH=TH, CAP=CAP, V=V, dinvP=dinvP, diagwP=diagwP,
                idx_w=idx_w, colP=colP, wfP=wfP)


def _host_golden(x, W1, b1, W2, b2, pp, out_dtype=np.float32, quant=True):
    """Numpy re-implementation of the device algorithm (same tiling, same
    bf16 quantization points). For validating the scheme off-device."""
    bf = (lambda a: a.astype(_BF16).astype(np.float32)) if quant else (lambda a: a)
    TH, CAP, V = pp["TH"], pp["CAP"], pp["V"]
    NB = NPAD // P

    dinv = pp["dinvP"].T.reshape(-1)
    diagw = pp["diagwP"].T.reshape(-1)

    xp = np.zeros((NPAD, FIN), np.float32)
    xp[:N] = x
    h1 = bf(xp) @ bf(W1)                     # bf16 inputs, f32 accum
    hs1 = bf(h1 * dinv[:, None])             # stored bf16

    idx_w = pp["idx_w"]
    colP = pp["colP"]
    wfP = np.asarray(pp["wfP"].astype(_BF16), dtype=np.float32)

    def agg(hs, F):
        out = np.zeros((NPAD, F), np.float32)
        for nb in range(NB):
            b = nb % BPC
            acc = (bf(diagw[nb * P:(nb + 1) * P])[:, None]
                   * hs[nb * P:(nb + 1) * P, :F])
            for hh in range(2):
                iw = idx_w[nb, hh, :16, :]                      # [16, IW]
                flat = iw.T.reshape(-1)[:CAP].astype(np.int64)  # unwrap
                base = 0 if hh == 0 else HALF
                for t in range(V[b, hh] // P):
                    oh = np.zeros((P, P), np.float32)
                    c = colP[:, (nb * 2 + hh) * TH + t]
                    wv = wfP[:, (nb * 2 + hh) * TH + t]
                    oh[np.arange(P), c.astype(np.int64)] = wv
                    msgs = hs[base + flat[t * P:(t + 1) * P]]
                    acc = acc + oh.T @ msgs
            out[nb * P:(nb + 1) * P] = acc
        return out

    out1 = agg(hs1, H) * dinv[:, None] + b1[None, :]
    out1 = np.maximum(out1, 0.0)

    h2 = bf(out1) @ bf(W2)
    hs2 = bf(h2 * dinv[:, None])
    out2 = agg(hs2, F2) * dinv[:, None] + b2[None, :]
    return out2[:N].astype(out_dtype)


# ---------------------------------------------------------------------------
# Bass device kernel
# ---------------------------------------------------------------------------

_NC_CACHE = {}


def _build_nc(TH, Vkey):
    import concourse.bass as bass  # noqa: F401
    import concourse.mybir as mybir
    import concourse.tile as tile
    from concourse import bacc
    from concourse.library_config import mlp

    DT = mybir.dt.bfloat16
    F32 = mybir.dt.float32
    I16 = mybir.dt.int16
    AL = mybir.AluOpType
    AF = mybir.ActivationFunctionType

    V = np.array(Vkey, dtype=np.int64).reshape(BPC, 2)
    CAP = TH * P
    IW = CAP // 16
    NB = NPAD // P           # 392

    nc = bacc.Bacc("TRN2", target_bir_lowering=False, debug=True,
                   num_devices=NCORES)
    xt3_d = nc.dram_tensor("xt3", [2, P, NPAD], DT, kind="ExternalInput")
    w1_d = nc.dram_tensor("w1c", [2, P, H], DT, kind="ExternalInput")
    w2_d = nc.dram_tensor("w2c", [2, P, F2], DT, kind="ExternalInput")
    b1_d = nc.dram_tensor("b1f", [P, H], F32, kind="ExternalInput")
    b2_d = nc.dram_tensor("b2f", [P, F2], F32, kind="ExternalInput")
    iota_d = nc.dram_tensor("iota", [P, P], DT, kind="ExternalInput")
    pidx_d = nc.dram_tensor("pidxf", [P, 1], F32, kind="ExternalInput")
    dinv_d = nc.dram_tensor("dinvP", [P, NB], F32, kind="ExternalInput")
    dinvl_d = nc.dram_tensor("dinvlP", [P, BPC], F32, kind="ExternalInput")
    diagwl_d = nc.dram_tensor("diagwlP", [P, BPC], F32, kind="ExternalInput")
    idx_d = nc.dram_tensor("idxP", [P, BPC * 2 * IW], I16, kind="ExternalInput")
    col_d = nc.dram_tensor("colP", [P, BPC * 2 * TH], F32, kind="ExternalInput")
    wf_d = nc.dram_tensor("wfP", [P, BPC * 2 * TH], F32, kind="ExternalInput")
    out_d = nc.dram_tensor("out2", [SHARD, F2], F32, kind="ExternalOutput")

    with tile.TileContext(nc) as tc:
        with (
            tc.tile_pool(name="dram", bufs=1, space="DRAM") as dpool,
            tc.tile_pool(name="const", bufs=1) as cpool,
            tc.tile_pool(name="xs", bufs=4) as xpool,
            tc.tile_pool(name="hst", bufs=4) as hpool,
            tc.tile_pool(name="msg", bufs=4) as mpool,
            tc.tile_pool(name="oh", bufs=16) as ohpool,
            tc.tile_pool(name="post", bufs=3) as tpool,
            tc.tile_pool(name="ph1", bufs=2, space="PSUM") as ph1p,
            tc.tile_pool(name="pagg", bufs=3, space="PSUM") as paggp,
            tc.tile_pool(name="pc", bufs=3, space="PSUM") as pcp,
        ):
            hs1_tab = dpool.tile([NPAD, H], DT)
            h2in_dram = dpool.tile([SHARD, H], DT)
            hs2_shard = dpool.tile([SHARD, F2], DT)
            hs2_full = dpool.tile([NPAD, F2], DT, addr_space="Shared")

            nc.gpsimd.load_library(mlp)

            # ---- constants ----
            w1_sb = cpool.tile([P, 2 * H], DT)
            nc.sync.dma_start(out=w1_sb[:, 0:H], in_=w1_d[0])
            nc.sync.dma_start(out=w1_sb[:, H:2 * H], in_=w1_d[1])
            w2_sb = cpool.tile([P, 2 * F2], DT)
            nc.sync.dma_start(out=w2_sb[:, 0:F2], in_=w2_d[0])
            nc.sync.dma_start(out=w2_sb[:, F2:2 * F2], in_=w2_d[1])
            b1_sb = cpool.tile([P, H], F32)
            nc.sync.dma_start(out=b1_sb[:], in_=b1_d[:])
            b2_sb = cpool.tile([P, F2], F32)
            nc.sync.dma_start(out=b2_sb[:], in_=b2_d[:])
            iota_sb = cpool.tile([P, P], DT)
            nc.sync.dma_start(out=iota_sb[:], in_=iota_d[:])
            pidx_sb = cpool.tile([P, 1], F32)
            nc.sync.dma_start(out=pidx_sb[:], in_=pidx_d[:])
            idx_sb = cpool.tile([P, BPC * 2 * IW], I16)
            nc.sync.dma_start(out=idx_sb[:], in_=idx_d[:])
            col_sb = cpool.tile([P, BPC * 2 * TH], F32)
            nc.sync.dma_start(out=col_sb[:], in_=col_d[:])
            wf_sb = cpool.tile([P, BPC * 2 * TH], F32)
            nc.sync.dma_start(out=wf_sb[:], in_=wf_d[:])
            dinv_sb = cpool.tile([P, NB], F32)
            nc.sync.dma_start(out=dinv_sb[:], in_=dinv_d[:])
            dinvl_sb = cpool.tile([P, BPC], F32)
            nc.sync.dma_start(out=dinvl_sb[:], in_=dinvl_d[:])
            diagwl_sb = cpool.tile([P, BPC], F32)
            nc.sync.dma_start(out=diagwl_sb[:], in_=diagwl_d[:])

            # ---- phase A: h1 = x @ W1 (all nodes), hs1 = h1 * dinv ----
            # two blocks share one PSUM bank ([P, 512] f32); paired writes
            for s in range(NPAD // 512):
                xa = xpool.tile([P, 512], DT, tag="xa")
                xb = xpool.tile([P, 512], DT, tag="xb")
                nc.sync.dma_start(out=xa[:], in_=xt3_d[0][:, s * 512:(s + 1) * 512])
                nc.sync.dma_start(out=xb[:], in_=xt3_d[1][:, s * 512:(s + 1) * 512])
                for hq in range(2):
                    nb0 = s * 4 + hq * 2
                    ph = ph1p.tile([P, 2 * H], F32)
                    for j in range(2):
                        q = hq * 2 + j
                        nc.tensor.matmul(ph[:, j * H:(j + 1) * H],
                                         lhsT=xa[:, q * P:(q + 1) * P],
                                         rhs=w1_sb[:, 0:H], start=True, stop=False)
                        nc.tensor.matmul(ph[:, j * H:(j + 1) * H],
                                         lhsT=xb[:, q * P:(q + 1) * P],
                                         rhs=w1_sb[:, H:2 * H], start=False,
                                         stop=True)
                    hst = hpool.tile([P, 2, H], DT, tag="hst")
                    for j in range(2):
                        nb = nb0 + j
                        if j == 0:
                            nc.scalar.activation(hst[:, j, :], ph[:, j * H:(j + 1) * H],
                                                 AF.Copy, scale=dinv_sb[:, nb:nb + 1])
                        else:
                            nc.vector.tensor_scalar(hst[:, j, :], ph[:, j * H:(j + 1) * H],
                                                    dinv_sb[:, nb:nb + 1], None,
                                                    AL.mult)
                    nc.sync.dma_start(
                        out=hs1_tab[nb0 * P:(nb0 + 2) * P, :].rearrange(
                            "(i p) f -> p i f", p=P),
                        in_=hst[:])

            pid_sc = nc.scalar.partition_id()

            def aggregate(b, tab, tab_own, F, psum_pool, psum_tag):
                """Edge aggregation for dest block b from table `tab` (+ the
                diagonal term from the core's own block). Returns PSUM tile."""
                msgs = []
                for hh in range(2):
                    nt = int(V[b, hh]) // P
                    m = mpool.tile([P, TH, F], DT, tag=f"msg{hh}")
                    src = tab[0:HALF, :] if hh == 0 else tab[HALF:NPAD, :]
                    nc.gpsimd.dma_gather(
                        m[:, 0:nt, :], src,
                        idx_sb[:, (b * 2 + hh) * IW:(b * 2 + hh) * IW + nt * 8],
                        int(V[b, hh]), int(V[b, hh]), F, single_packet=False)
                    msgs.append(m)
                own = ohpool.tile([P, F], DT, tag="own")
                if tab_own is None:
                    nc.scalar.dma_start(
                        out=own[:],
                        in_=tab[bass.ds((pid_sc * BPC + b) * P, P), 0:F])
                else:
                    nc.scalar.dma_start(
                        out=own[:], in_=tab_own[b * P:(b + 1) * P, 0:F])
                dg = ohpool.tile([P, P], DT, tag="oh")
                nc.vector.tensor_scalar(dg[:], iota_sb[:], pidx_sb[:, 0:1],
                                        diagwl_sb[:, b:b + 1], AL.is_equal,
                                        AL.mult)
                pagg = psum_pool.tile([P, F], F32, tag=psum_tag)
                t = 0
                for hh in range(2):
                    for tt in range(int(V[b, hh]) // P):
                        oh = ohpool.tile([P, P], DT, tag="oh")
                        sc = (b * 2 + hh) * TH + tt
                        nc.vector.tensor_scalar(oh[:], iota_sb[:],
                                                col_sb[:, sc:sc + 1],
                                                wf_sb[:, sc:sc + 1],
                                                AL.is_equal, AL.mult)
                        nc.tensor.matmul(pagg[:], lhsT=oh[:],
                                         rhs=msgs[hh][:, tt, :],
                                         start=(t == 0), stop=False)
                        t += 1
                nc.tensor.matmul(pagg[:], lhsT=dg[:], rhs=own[:],
                                 start=False, stop=True)
                return pagg

            # ---- phase B: layer-1 aggregation per dest block, with the
            #      layer-2 projection (relu_out @ W2) interleaved ----
            for b in range(BPC):
                pagg = aggregate(b, hs1_tab, None, H, paggp, "")
                t1 = tpool.tile([P, H], F32, tag="t1")
                nc.vector.tensor_scalar(t1[:], pagg[:], dinvl_sb[:, b:b + 1], None,
                                        AL.mult)
                t2 = tpool.tile([P, H], F32, tag="t2")
                nc.vector.tensor_tensor(t2[:], t1[:], b1_sb[:], AL.add)
                rl = hpool.tile([P, H], DT, tag="rl")
                nc.scalar.activation(rl[:], t2[:], AF.Relu)
                nc.scalar.dma_start(out=h2in_dram[b * P:(b + 1) * P, :], in_=rl[:])

                # layer-2 projection for this block
                ph2 = pcp.tile([P, F2], F32, tag="pc")
                for c2 in range(2):
                    at = ohpool.tile([P, P], DT, tag="at")
                    nc.scalar.dma_start(
                        out=at[:],
                        in_=h2in_dram[b * P:(b + 1) * P, c2 * P:(c2 + 1) * P],
                        transpose=True)
                    nc.tensor.matmul(ph2[:], lhsT=at[:],
                                     rhs=w2_sb[:, c2 * F2:(c2 + 1) * F2],
                                     start=(c2 == 0), stop=(c2 == 1))
                hsb = hpool.tile([P, F2], DT, tag="hsb")
                nc.scalar.activation(hsb[:], ph2[:], AF.Copy,
                                     scale=dinvl_sb[:, b:b + 1])
                nc.scalar.dma_start(out=hs2_shard[b * P:(b + 1) * P, :], in_=hsb[:])

            # ---- phase D: exchange hs2 shards ----
            nc.gpsimd.collective_compute(
                "AllGather", AL.bypass,
                replica_groups=[list(range(NCORES))],
                ins=[hs2_shard[:]],
                outs=[hs2_full[:]],
            )

            # ---- phase E: layer-2 aggregation per dest block ----
            for b in range(BPC):
                pagg2 = aggregate(b, hs2_full, hs2_shard, F2, pcp, "pc")
                o1 = tpool.tile([P, F2], F32, tag="o1")
                nc.vector.tensor_scalar(o1[:], pagg2[:], dinvl_sb[:, b:b + 1], None,
                                        AL.mult)
                o2 = tpool.tile([P, F2], F32, tag="o2")
                nc.vector.tensor_tensor(o2[:], o1[:], b2_sb[:], AL.add)
                nc.sync.dma_start(out=out_d[b * P:(b + 1) * P, :], in_=o2[:])

    nc.compile()
    return nc


def _make_inputs(x, W1, b1, W2, b2, pp):
    """Per-core input maps."""
    TH = pp["TH"]
    IW = (TH * P) // 16
    NB = NPAD // P

    xp = np.zeros((NPAD, FIN), np.float32)
    xp[:N] = x
    xt3 = np.ascontiguousarray(
        xp.T.reshape(2, P, NPAD).astype(_BF16))
    w1c = np.ascontiguousarray(W1.reshape(2, P, H).astype(_BF16))
    w2c = np.ascontiguousarray(W2.reshape(2, P, F2).astype(_BF16))
    b1f = np.ascontiguousarray(np.tile(b1[None, :], (P, 1)).astype(np.float32))
    b2f = np.ascontiguousarray(np.tile(b2[None, :], (P, 1)).astype(np.float32))
    iota = np.tile(np.arange(P, dtype=np.float32)[None, :], (P, 1)).astype(_BF16)
    pidxf = np.arange(P, dtype=np.float32)[:, None].copy()

    dinvP = pp["dinvP"]
    diagwP = pp["diagwP"]
    idx_w = pp["idx_w"]        # [NB, 2, 128, IW]
    colP = pp["colP"]          # [128, NB*2*TH]
    wfP = pp["wfP"]

    in_maps = []
    for c in range(NCORES):
        b0 = c * BPC
        idxP = np.ascontiguousarray(
            idx_w[b0:b0 + BPC].transpose(2, 0, 1, 3).reshape(P, BPC * 2 * IW))
        in_maps.append({
            "xt3": xt3,
            "w1c": w1c,
            "w2c": w2c,
            "b1f": b1f,
            "b2f": b2f,
            "iota": iota,
            "pidxf": pidxf,
            "dinvP": dinvP,
            "dinvlP": np.ascontiguousarray(dinvP[:, b0:b0 + BPC]),
            "diagwlP": np.ascontiguousarray(diagwP[:, b0:b0 + BPC]),
            "idxP": idxP,
            "colP": np.ascontiguousarray(colP[:, b0 * 2 * TH:(b0 + BPC) * 2 * TH]),
            "wfP": np.ascontiguousarray(wfP[:, b0 * 2 * TH:(b0 + BPC) * 2 * TH]),
        })
    return in_maps


def kernel(x, edge_index, edge_weight, W1, b1, W2, b2, _trace=False):
    from concourse.bass_utils import run_bass_kernel_spmd

    x = np.asarray(x, dtype=np.float32)
    W1 = np.asarray(W1, dtype=np.float32)
    b1 = np.asarray(b1, dtype=np.float32)
    W2 = np.asarray(W2, dtype=np.float32)
    b2 = np.asarray(b2, dtype=np.float32)

    pp = _preprocess(np.asarray(edge_index), np.asarray(edge_weight))
    key = (pp["TH"], tuple(pp["V"].reshape(-1).tolist()))
    if key not in _NC_CACHE:
        _NC_CACHE[key] = _build_nc(*key)
    nc = _NC_CACHE[key]

    in_maps = _make_inputs(x, W1, b1, W2, b2, pp)
    res = run_bass_kernel_spmd(nc, in_maps, list(range(NCORES)), trace=_trace)
    out = np.concatenate([res.results[c]["out2"] for c in range(NCORES)], axis=0)
    if _trace:
        kernel._last_result = res
    return np.ascontiguousarray(out[:N])


# revision 11
# speedup vs baseline: 1.4155x; 1.1234x over previous
"""GCN encoder (2-layer) Bass kernel for Trainium2, 8 NeuronCores.

Strategy (graph/data parallel, per sharding hint):
  - Nodes padded to NPAD=50176 and sharded by contiguous range: core c owns
    destination nodes [c*6272, (c+1)*6272) = 49 blocks of 128.
  - Edges (incl. self-loops) are bucketed by destination block and by source
    half (dma_gather indices are int16, so the feature table is gathered in
    two halves of 25088 rows each), sorted by source within each bucket for
    HBM page locality.  Edges with src==dst ("diagonal" edges, incl. all
    self-loops) are pulled out of the buckets and handled by one per-block
    diagonal matmul instead (no gather needed).  Each (block,half) bucket is
    padded to V = max-over-cores ceil128(count) so all 8 cores run one
    identical SPMD program with per-gather-exact descriptor counts (the Q7
    descriptor-generation on the GpSimd engine is the kernel bottleneck).
  - Per layer: h = x @ W (dense matmul, PSUM f32), table hs = h * dinv[src]
    stored in HBM (bf16); per destination block, edge messages are fetched
    with dma_gather (128 edges/tile, edge-major) and segment-summed on the
    TensorEngine via one-hot matmuls: onehot[k,d] = w[k] * (col[k]==d), so
    PSUM[d,f] += sum_k w[k]*hs[src_k][f]. Post: * dinv[dest] + bias (+relu).
  - The layer-2 projection (relu_out @ W2) is interleaved into the layer-1
    block loop; hs2 shards are then exchanged with an AllGather collective.
  - dinv = rsqrt(deg) is precomputed on the host (graph-structure preproc).

kernel(**inputs) takes the FULL inputs and returns the FULL [50000,128] f32
output; all sharding/gather happens inside.
"""

import sys

sys.path.insert(0, "/opt/trn_rl_repo")

import numpy as np
import ml_dtypes

P = 128
NCORES = 8
BPC = 49                 # dest blocks per core
SHARD = BPC * P          # 6272
NPAD = NCORES * SHARD    # 50176
HALF = NPAD // 2         # 25088
N = 50000
FIN = 256
H = 256                  # layer-1 output width
F2 = 128                 # layer-2 output width
DUMMY_SRC = N + 8        # a zero (pad) node, used as src for pad edges

_BF16 = ml_dtypes.bfloat16


def _preprocess(edge_index, edge_weight):
    """Build all per-core device input arrays from the edge list."""
    row = np.asarray(edge_index[0], dtype=np.int64)
    col = np.asarray(edge_index[1], dtype=np.int64)
    w = np.asarray(edge_weight, dtype=np.float32)

    loop = np.arange(N, dtype=np.int64)
    rows = np.concatenate([row, loop])
    cols = np.concatenate([col, loop])
    ws = np.concatenate([w, np.ones(N, np.float32)])

    # ---- dinv on host (graph-structure preprocessing) ----
    deg = np.zeros(NPAD, np.float32)
    np.add.at(deg, cols, ws)
    deg = deg + (deg == 0)
    dinv = np.sqrt(1.0 / deg).astype(np.float32)
    # partition-major [128, 392]: dinvP[p, nb] = dinv[nb*128+p]
    dinvP = np.ascontiguousarray(dinv.reshape(NPAD // P, P).T)

    # ---- diagonal (src==dst) edges -> per-node diag weight ----
    isdiag = rows == cols
    diagw = np.zeros(NPAD, np.float32)
    np.add.at(diagw, cols[isdiag], ws[isdiag])
    diagwP = np.ascontiguousarray(diagw.reshape(NPAD // P, P).T)

    rows = rows[~isdiag]
    cols = cols[~isdiag]
    ws = ws[~isdiag]
    EE = rows.shape[0]

    # ---- edge streams per (block, half), sorted by src within bucket ----
    blk = cols // P                      # 0..391
    half = (rows >= HALF).astype(np.int64)
    key = blk * 2 + half
    cnt = np.bincount(key, minlength=(NPAD // P) * 2)
    TH = int(-(-cnt.max() // P))         # tiles per half (max over all)
    CAP = TH * P
    NB = NPAD // P                       # 392 blocks

    src_a = np.full((NB, 2, CAP), DUMMY_SRC % HALF, np.int16)
    col_a = np.zeros((NB, 2, CAP), np.float32)
    w_a = np.zeros((NB, 2, CAP), np.float32)

    # sort by (bucket, src) so gathered rows are ascending within a bucket
    order2 = np.lexsort((rows, key))
    cs2 = np.zeros(NB * 2 + 1, np.int64)
    np.cumsum(cnt, out=cs2[1:])
    pos = np.arange(EE) - cs2[key[order2]]
    kb = key[order2] // 2
    kh = key[order2] % 2
    src_sorted = rows[order2]
    src_rel = np.where(kh == 1, src_sorted - HALF, src_sorted).astype(np.int16)
    src_a[kb, kh, pos] = src_rel
    col_a[kb, kh, pos] = (cols[order2] - kb * P).astype(np.float32)
    w_a[kb, kh, pos] = ws[order2]

    # per-(block,half) gather length: max over the 8 cores at the same block
    # position of ceil128(count); identical across cores -> static program.
    cnt2 = cnt.reshape(NCORES, BPC, 2)
    tcnt = -(-cnt2 // P)                       # tiles, [8, 49, 2]
    tmax = tcnt.max(axis=0)                    # [49, 2]
    V = tmax * P                               # gather num_idxs per position

    # wrapped int16 index layout for dma_gather: index i -> partition i%16,
    # col i//16, replicated across the 8 groups of 16 partitions.
    IW = CAP // 16
    idx_w = src_a.reshape(NB, 2, IW, 16).transpose(0, 1, 3, 2)  # [NB,2,16,IW]
    idx_w = np.ascontiguousarray(np.tile(idx_w, (1, 1, 8, 1)))  # [NB,2,128,IW]

    # col/w in per-tile scalar layout: [.., 128, 2*TH] where slot (h*TH+t)
    # on partition p = edge t*128+p of half h.
    colP = col_a.reshape(NB, 2, TH, P).transpose(3, 0, 1, 2).reshape(P, NB * 2 * TH)
    wfP = w_a.reshape(NB, 2, TH, P).transpose(3, 0, 1, 2).reshape(P, NB * 2 * TH)
    colP = np.ascontiguousarray(colP)
    wfP = np.ascontiguousarray(wfP)

    return dict(TH=TH, CAP=CAP, V=V, dinvP=dinvP, diagwP=diagwP,
                idx_w=idx_w, colP=colP, wfP=wfP)


def _host_golden(x, W1, b1, W2, b2, pp, out_dtype=np.float32, quant=True):
    """Numpy re-implementation of the device algorithm (same tiling, same
    bf16 quantization points). For validating the scheme off-device."""
    bf = (lambda a: a.astype(_BF16).astype(np.float32)) if quant else (lambda a: a)
    TH, CAP, V = pp["TH"], pp["CAP"], pp["V"]
    NB = NPAD // P

    dinv = pp["dinvP"].T.reshape(-1)
    diagw = pp["diagwP"].T.reshape(-1)

    xp = np.zeros((NPAD, FIN), np.float32)
    xp[:N] = x
    h1 = bf(xp) @ bf(W1)                     # bf16 inputs, f32 accum
    hs1 = bf(h1 * dinv[:, None])             # stored bf16

    idx_w = pp["idx_w"]
    colP = pp["colP"]
    wfP = np.asarray(pp["wfP"].astype(_BF16), dtype=np.float32)

    def agg(hs, F):
        out = np.zeros((NPAD, F), np.float32)
        for nb in range(NB):
            b = nb % BPC
            acc = (bf(diagw[nb * P:(nb + 1) * P])[:, None]
                   * hs[nb * P:(nb + 1) * P, :F])
            for hh in range(2):
                iw = idx_w[nb, hh, :16, :]                      # [16, IW]
                flat = iw.T.reshape(-1)[:CAP].astype(np.int64)  # unwrap
                base = 0 if hh == 0 else HALF
                for t in range(V[b, hh] // P):
                    oh = np.zeros((P, P), np.float32)
                    c = colP[:, (nb * 2 + hh) * TH + t]
                    wv = wfP[:, (nb * 2 + hh) * TH + t]
                    oh[np.arange(P), c.astype(np.int64)] = wv
                    msgs = hs[base + flat[t * P:(t + 1) * P]]
                    acc = acc + oh.T @ msgs
            out[nb * P:(nb + 1) * P] = acc
        return out

    out1 = agg(hs1, H) * dinv[:, None] + b1[None, :]
    out1 = np.maximum(out1, 0.0)

    h2 = bf(out1) @ bf(W2)
    hs2 = bf(h2 * dinv[:, None])
    out2 = agg(hs2, F2) * dinv[:, None] + b2[None, :]
    return out2[:N].astype(out_dtype)


# ---------------------------------------------------------------------------
# Bass device kernel
# ---------------------------------------------------------------------------

_NC_CACHE = {}


def _build_nc(TH, Vkey):
    import concourse.bass as bass  # noqa: F401
    import concourse.mybir as mybir
    import concourse.tile as tile
    from concourse import bacc
    from concourse.library_config import mlp

    DT = mybir.dt.bfloat16
    F32 = mybir.dt.float32
    I16 = mybir.dt.int16
    AL = mybir.AluOpType
    AF = mybir.ActivationFunctionType

    V = np.array(Vkey, dtype=np.int64).reshape(BPC, 2)
    CAP = TH * P
    IW = CAP // 16
    NB = NPAD // P           # 392

    nc = bacc.Bacc("TRN2", target_bir_lowering=False, debug=True,
                   num_devices=NCORES, num_swdge_queues=4)
    xt3_d = nc.dram_tensor("xt3", [2, P, NPAD], DT, kind="ExternalInput")
    w1_d = nc.dram_tensor("w1c", [2, P, H], DT, kind="ExternalInput")
    w2_d = nc.dram_tensor("w2c", [2, P, F2], DT, kind="ExternalInput")
    b1_d = nc.dram_tensor("b1f", [P, H], F32, kind="ExternalInput")
    b2_d = nc.dram_tensor("b2f", [P, F2], F32, kind="ExternalInput")
    iota_d = nc.dram_tensor("iota", [P, P], DT, kind="ExternalInput")
    pidx_d = nc.dram_tensor("pidxf", [P, 1], F32, kind="ExternalInput")
    dinv_d = nc.dram_tensor("dinvP", [P, NB], F32, kind="ExternalInput")
    dinvl_d = nc.dram_tensor("dinvlP", [P, BPC], F32, kind="ExternalInput")
    diagwl_d = nc.dram_tensor("diagwlP", [P, BPC], F32, kind="ExternalInput")
    idx_d = nc.dram_tensor("idxP", [P, BPC * 2 * IW], I16, kind="ExternalInput")
    col_d = nc.dram_tensor("colP", [P, BPC * 2 * TH], F32, kind="ExternalInput")
    wf_d = nc.dram_tensor("wfP", [P, BPC * 2 * TH], F32, kind="ExternalInput")
    out_d = nc.dram_tensor("out2", [SHARD, F2], F32, kind="ExternalOutput")

    with tile.TileContext(nc) as tc:
        with (
            tc.tile_pool(name="dram", bufs=1, space="DRAM") as dpool,
            tc.tile_pool(name="const", bufs=1) as cpool,
            tc.tile_pool(name="xs", bufs=4) as xpool,
            tc.tile_pool(name="hst", bufs=4) as hpool,
            tc.tile_pool(name="msg", bufs=4) as mpool,
            tc.tile_pool(name="oh", bufs=16) as ohpool,
            tc.tile_pool(name="post", bufs=3) as tpool,
            tc.tile_pool(name="ph1", bufs=3, space="PSUM") as ph1p,
            tc.tile_pool(name="pagg", bufs=3, space="PSUM") as paggp,
            tc.tile_pool(name="pc", bufs=2, space="PSUM") as pcp,
        ):
            hs1_tab = dpool.tile([NPAD, H], DT)
            h2in_dram = dpool.tile([SHARD, H], DT)
            hs2_shard = dpool.tile([SHARD, F2], DT)
            hs2_full = dpool.tile([NPAD, F2], DT, addr_space="Shared")

            nc.gpsimd.load_library(mlp)

            # ---- constants ----
            w1_sb = cpool.tile([P, 2 * H], DT)
            nc.sync.dma_start(out=w1_sb[:, 0:H], in_=w1_d[0])
            nc.sync.dma_start(out=w1_sb[:, H:2 * H], in_=w1_d[1])
            w2_sb = cpool.tile([P, 2 * F2], DT)
            nc.sync.dma_start(out=w2_sb[:, 0:F2], in_=w2_d[0])
            nc.sync.dma_start(out=w2_sb[:, F2:2 * F2], in_=w2_d[1])
            b1_sb = cpool.tile([P, H], F32)
            nc.sync.dma_start(out=b1_sb[:], in_=b1_d[:])
            b2_sb = cpool.tile([P, F2], F32)
            nc.sync.dma_start(out=b2_sb[:], in_=b2_d[:])
            iota_sb = cpool.tile([P, P], DT)
            nc.sync.dma_start(out=iota_sb[:], in_=iota_d[:])
            pidx_sb = cpool.tile([P, 1], F32)
            nc.sync.dma_start(out=pidx_sb[:], in_=pidx_d[:])
            idx_sb = cpool.tile([P, BPC * 2 * IW], I16)
            nc.sync.dma_start(out=idx_sb[:], in_=idx_d[:])
            col_sb = cpool.tile([P, BPC * 2 * TH], F32)
            nc.sync.dma_start(out=col_sb[:], in_=col_d[:])
            wf_sb = cpool.tile([P, BPC * 2 * TH], F32)
            nc.sync.dma_start(out=wf_sb[:], in_=wf_d[:])
            dinv_sb = cpool.tile([P, NB], F32)
            nc.sync.dma_start(out=dinv_sb[:], in_=dinv_d[:])
            dinvl_sb = cpool.tile([P, BPC], F32)
            nc.sync.dma_start(out=dinvl_sb[:], in_=dinvl_d[:])
            diagwl_sb = cpool.tile([P, BPC], F32)
            nc.sync.dma_start(out=diagwl_sb[:], in_=diagwl_d[:])

            # ---- phase A: h1 = x @ W1 (all nodes), hs1 = h1 * dinv ----
            # two blocks share one PSUM bank ([P, 512] f32); paired writes
            for s in range(NPAD // 512):
                xa = xpool.tile([P, 512], DT, tag="xa")
                xb = xpool.tile([P, 512], DT, tag="xb")
                nc.sync.dma_start(out=xa[:], in_=xt3_d[0][:, s * 512:(s + 1) * 512])
                nc.sync.dma_start(out=xb[:], in_=xt3_d[1][:, s * 512:(s + 1) * 512])
                for hq in range(2):
                    nb0 = s * 4 + hq * 2
                    ph = ph1p.tile([P, 2 * H], F32)
                    for j in range(2):
                        q = hq * 2 + j
                        nc.tensor.matmul(ph[:, j * H:(j + 1) * H],
                                         lhsT=xa[:, q * P:(q + 1) * P],
                                         rhs=w1_sb[:, 0:H], start=True, stop=False)
                        nc.tensor.matmul(ph[:, j * H:(j + 1) * H],
                                         lhsT=xb[:, q * P:(q + 1) * P],
                                         rhs=w1_sb[:, H:2 * H], start=False,
                                         stop=True)
                    hst = hpool.tile([P, 2, H], DT, tag="hst")
                    for j in range(2):
                        nb = nb0 + j
                        if j == 0:
                            nc.scalar.activation(hst[:, j, :], ph[:, j * H:(j + 1) * H],
                                                 AF.Copy, scale=dinv_sb[:, nb:nb + 1])
                        else:
                            nc.vector.tensor_scalar(hst[:, j, :], ph[:, j * H:(j + 1) * H],
                                                    dinv_sb[:, nb:nb + 1], None,
                                                    AL.mult)
                    nc.sync.dma_start(
                        out=hs1_tab[nb0 * P:(nb0 + 2) * P, :].rearrange(
                            "(i p) f -> p i f", p=P),
                        in_=hst[:])

            pid_sc = nc.scalar.partition_id()

            def aggregate(b, tab, tab_own, F, psum_pool, psum_tag):
                """Edge aggregation for dest block b from table `tab` (+ the
                diagonal term from the core's own block). Returns PSUM tile."""
                msgs = []
                for hh in range(2):
                    nt = int(V[b, hh]) // P
                    m = mpool.tile([P, TH, F], DT, tag=f"msg{hh}")
                    src = tab[0:HALF, :] if hh == 0 else tab[HALF:NPAD, :]
                    nc.gpsimd.dma_gather(
                        m[:, 0:nt, :], src,
                        idx_sb[:, (b * 2 + hh) * IW:(b * 2 + hh) * IW + nt * 8],
                        int(V[b, hh]), int(V[b, hh]), F, single_packet=False,
                        queue_num=(b * 2 + hh) % 4)
                    msgs.append(m)
                own = ohpool.tile([P, F], DT, tag="own")
                if tab_own is None:
                    nc.scalar.dma_start(
                        out=own[:],
                        in_=tab[bass.ds((pid_sc * BPC + b) * P, P), 0:F])
                else:
                    nc.scalar.dma_start(
                        out=own[:], in_=tab_own[b * P:(b + 1) * P, 0:F])
                dg = ohpool.tile([P, P], DT, tag="oh")
                nc.vector.tensor_scalar(dg[:], iota_sb[:], pidx_sb[:, 0:1],
                                        diagwl_sb[:, b:b + 1], AL.is_equal,
                                        AL.mult)
                pagg = psum_pool.tile([P, F], F32, tag=psum_tag)
                t = 0
                for hh in range(2):
                    for tt in range(int(V[b, hh]) // P):
                        oh = ohpool.tile([P, P], DT, tag="oh")
                        sc = (b * 2 + hh) * TH + tt
                        nc.vector.tensor_scalar(oh[:], iota_sb[:],
                                                col_sb[:, sc:sc + 1],
                                                wf_sb[:, sc:sc + 1],
                                                AL.is_equal, AL.mult)
                        nc.tensor.matmul(pagg[:], lhsT=oh[:],
                                         rhs=msgs[hh][:, tt, :],
                                         start=(t == 0), stop=False)
                        t += 1
                nc.tensor.matmul(pagg[:], lhsT=dg[:], rhs=own[:],
                                 start=False, stop=True)
                return pagg

            # ---- phase B: layer-1 aggregation per dest block, with the
            #      layer-2 projection (relu_out @ W2) interleaved ----
            for b in range(BPC):
                pagg = aggregate(b, hs1_tab, None, H, paggp, "")
                t1 = tpool.tile([P, H], F32, tag="t1")
                nc.vector.tensor_scalar(t1[:], pagg[:], dinvl_sb[:, b:b + 1], None,
                                        AL.mult)
                t2 = tpool.tile([P, H], F32, tag="t2")
                nc.vector.tensor_tensor(t2[:], t1[:], b1_sb[:], AL.add)
                rl = hpool.tile([P, H], DT, tag="rl")
                nc.scalar.activation(rl[:], t2[:], AF.Relu)
                nc.scalar.dma_start(out=h2in_dram[b * P:(b + 1) * P, :], in_=rl[:])

                # layer-2 projection for this block
                ph2 = pcp.tile([P, F2], F32, tag="pc")
                for c2 in range(2):
                    at = ohpool.tile([P, P], DT, tag="at")
                    nc.scalar.dma_start(
                        out=at[:],
                        in_=h2in_dram[b * P:(b + 1) * P, c2 * P:(c2 + 1) * P],
                        transpose=True)
                    nc.tensor.matmul(ph2[:], lhsT=at[:],
                                     rhs=w2_sb[:, c2 * F2:(c2 + 1) * F2],
                                     start=(c2 == 0), stop=(c2 == 1))
                hsb = hpool.tile([P, F2], DT, tag="hsb")
                nc.scalar.activation(hsb[:], ph2[:], AF.Copy,
                                     scale=dinvl_sb[:, b:b + 1])
                nc.scalar.dma_start(out=hs2_shard[b * P:(b + 1) * P, :], in_=hsb[:])

            # ---- phase D: exchange hs2 shards ----
            nc.gpsimd.collective_compute(
                "AllGather", AL.bypass,
                replica_groups=[list(range(NCORES))],
                ins=[hs2_shard[:]],
                outs=[hs2_full[:]],
            )

            # ---- phase E: layer-2 aggregation per dest block ----
            for b in range(BPC):
                pagg2 = aggregate(b, hs2_full, hs2_shard, F2, pcp, "pc")
                o1 = tpool.tile([P, F2], F32, tag="o1")
                nc.vector.tensor_scalar(o1[:], pagg2[:], dinvl_sb[:, b:b + 1], None,
                                        AL.mult)
                o2 = tpool.tile([P, F2], F32, tag="o2")
                nc.vector.tensor_tensor(o2[:], o1[:], b2_sb[:], AL.add)
                nc.sync.dma_start(out=out_d[b * P:(b + 1) * P, :], in_=o2[:])

    nc.compile()
    return nc


def _make_inputs(x, W1, b1, W2, b2, pp):
    """Per-core input maps."""
    TH = pp["TH"]
    IW = (TH * P) // 16
    NB = NPAD // P

    xp = np.zeros((NPAD, FIN), np.float32)
    xp[:N] = x
    xt3 = np.ascontiguousarray(
        xp.T.reshape(2, P, NPAD).astype(_BF16))
    w1c = np.ascontiguousarray(W1.reshape(2, P, H).astype(_BF16))
    w2c = np.ascontiguousarray(W2.reshape(2, P, F2).astype(_BF16))
    b1f = np.ascontiguousarray(np.tile(b1[None, :], (P, 1)).astype(np.float32))
    b2f = np.ascontiguousarray(np.tile(b2[None, :], (P, 1)).astype(np.float32))
    iota = np.tile(np.arange(P, dtype=np.float32)[None, :], (P, 1)).astype(_BF16)
    pidxf = np.arange(P, dtype=np.float32)[:, None].copy()

    dinvP = pp["dinvP"]
    diagwP = pp["diagwP"]
    idx_w = pp["idx_w"]        # [NB, 2, 128, IW]
    colP = pp["colP"]          # [128, NB*2*TH]
    wfP = pp["wfP"]

    in_maps = []
    for c in range(NCORES):
        b0 = c * BPC
        idxP = np.ascontiguousarray(
            idx_w[b0:b0 + BPC].transpose(2, 0, 1, 3).reshape(P, BPC * 2 * IW))
        in_maps.append({
            "xt3": xt3,
            "w1c": w1c,
            "w2c": w2c,
            "b1f": b1f,
            "b2f": b2f,
            "iota": iota,
            "pidxf": pidxf,
            "dinvP": dinvP,
            "dinvlP": np.ascontiguousarray(dinvP[:, b0:b0 + BPC]),
            "diagwlP": np.ascontiguousarray(diagwP[:, b0:b0 + BPC]),
            "idxP": idxP,
            "colP": np.ascontiguousarray(colP[:, b0 * 2 * TH:(b0 + BPC) * 2 * TH]),
            "wfP": np.ascontiguousarray(wfP[:, b0 * 2 * TH:(b0 + BPC) * 2 * TH]),
        })
    return in_maps


def kernel(x, edge_index, edge_weight, W1, b1, W2, b2, _trace=False):
    from concourse.bass_utils import run_bass_kernel_spmd

    x = np.asarray(x, dtype=np.float32)
    W1 = np.asarray(W1, dtype=np.float32)
    b1 = np.asarray(b1, dtype=np.float32)
    W2 = np.asarray(W2, dtype=np.float32)
    b2 = np.asarray(b2, dtype=np.float32)

    pp = _preprocess(np.asarray(edge_index), np.asarray(edge_weight))
    key = (pp["TH"], tuple(pp["V"].reshape(-1).tolist()))
    if key not in _NC_CACHE:
        _NC_CACHE[key] = _build_nc(*key)
    nc = _NC_CACHE[key]

    in_maps = _make_inputs(x, W1, b1, W2, b2, pp)
    res = run_bass_kernel_spmd(nc, in_maps, list(range(NCORES)), trace=_trace)
    out = np.concatenate([res.results[c]["out2"] for c in range(NCORES)], axis=0)
    if _trace:
        kernel._last_result = res
    return np.ascontiguousarray(out[:N])


# revision 12
# speedup vs baseline: 1.4444x; 1.0204x over previous
"""GCN encoder (2-layer) Bass kernel for Trainium2, 8 NeuronCores.

Strategy (graph/data parallel, per sharding hint):
  - Nodes padded to NPAD=50176 and sharded by contiguous range: core c owns
    destination nodes [c*6272, (c+1)*6272) = 49 blocks of 128.
  - Edges (incl. self-loops) are bucketed by destination block and by source
    half (dma_gather indices are int16, so the feature table is gathered in
    two halves of 25088 rows each), sorted by source within each bucket for
    HBM page locality.  Edges with src==dst ("diagonal" edges, incl. all
    self-loops) are pulled out of the buckets and handled by one per-block
    diagonal matmul instead (no gather needed).  Each (block,half) bucket is
    padded to V = max-over-cores ceil128(count) so all 8 cores run one
    identical SPMD program with per-gather-exact descriptor counts (the Q7
    descriptor-generation on the GpSimd engine is the kernel bottleneck).
  - Per layer: h = x @ W (dense matmul, PSUM f32), table hs = h * dinv[src]
    stored in HBM (bf16); per destination block, edge messages are fetched
    with dma_gather (128 edges/tile, edge-major) and segment-summed on the
    TensorEngine via one-hot matmuls: onehot[k,d] = w[k] * (col[k]==d), so
    PSUM[d,f] += sum_k w[k]*hs[src_k][f]. Post: * dinv[dest] + bias (+relu).
  - The layer-2 projection (relu_out @ W2) is interleaved into the layer-1
    block loop; hs2 shards are then exchanged with an AllGather collective.
  - dinv = rsqrt(deg) is precomputed on the host (graph-structure preproc).

kernel(**inputs) takes the FULL inputs and returns the FULL [50000,128] f32
output; all sharding/gather happens inside.
"""

import sys

sys.path.insert(0, "/opt/trn_rl_repo")

import numpy as np
import ml_dtypes

P = 128
NCORES = 8
BPC = 49                 # dest blocks per core
SHARD = BPC * P          # 6272
NPAD = NCORES * SHARD    # 50176
HALF = NPAD // 2         # 25088
N = 50000
FIN = 256
H = 256                  # layer-1 output width
F2 = 128                 # layer-2 output width
DUMMY_SRC = N + 8        # a zero (pad) node, used as src for pad edges

_BF16 = ml_dtypes.bfloat16


def _preprocess(edge_index, edge_weight):
    """Build all per-core device input arrays from the edge list."""
    row = np.asarray(edge_index[0], dtype=np.int64)
    col = np.asarray(edge_index[1], dtype=np.int64)
    w = np.asarray(edge_weight, dtype=np.float32)

    loop = np.arange(N, dtype=np.int64)
    rows = np.concatenate([row, loop])
    cols = np.concatenate([col, loop])
    ws = np.concatenate([w, np.ones(N, np.float32)])

    # ---- dinv on host (graph-structure preprocessing) ----
    deg = np.zeros(NPAD, np.float32)
    np.add.at(deg, cols, ws)
    deg = deg + (deg == 0)
    dinv = np.sqrt(1.0 / deg).astype(np.float32)
    # partition-major [128, 392]: dinvP[p, nb] = dinv[nb*128+p]
    dinvP = np.ascontiguousarray(dinv.reshape(NPAD // P, P).T)

    # ---- diagonal (src==dst) edges -> per-node diag weight ----
    isdiag = rows == cols
    diagw = np.zeros(NPAD, np.float32)
    np.add.at(diagw, cols[isdiag], ws[isdiag])
    diagwP = np.ascontiguousarray(diagw.reshape(NPAD // P, P).T)

    rows = rows[~isdiag]
    cols = cols[~isdiag]
    ws = ws[~isdiag]
    EE = rows.shape[0]

    # ---- edge streams per (block, half), sorted by src within bucket ----
    blk = cols // P                      # 0..391
    half = (rows >= HALF).astype(np.int64)
    key = blk * 2 + half
    cnt = np.bincount(key, minlength=(NPAD // P) * 2)
    TH = int(-(-cnt.max() // P))         # tiles per half (max over all)
    CAP = TH * P
    NB = NPAD // P                       # 392 blocks

    src_a = np.full((NB, 2, CAP), DUMMY_SRC % HALF, np.int16)
    col_a = np.zeros((NB, 2, CAP), np.float32)
    w_a = np.zeros((NB, 2, CAP), np.float32)

    # sort by (bucket, src) so gathered rows are ascending within a bucket
    order2 = np.lexsort((rows, key))
    cs2 = np.zeros(NB * 2 + 1, np.int64)
    np.cumsum(cnt, out=cs2[1:])
    pos = np.arange(EE) - cs2[key[order2]]
    kb = key[order2] // 2
    kh = key[order2] % 2
    src_sorted = rows[order2]
    src_rel = np.where(kh == 1, src_sorted - HALF, src_sorted).astype(np.int16)
    src_a[kb, kh, pos] = src_rel
    col_a[kb, kh, pos] = (cols[order2] - kb * P).astype(np.float32)
    w_a[kb, kh, pos] = ws[order2]

    # per-(block,half) gather length: max over the 8 cores at the same block
    # position of ceil128(count); identical across cores -> static program.
    cnt2 = cnt.reshape(NCORES, BPC, 2)
    tcnt = -(-cnt2 // P)                       # tiles, [8, 49, 2]
    tmax = tcnt.max(axis=0)                    # [49, 2]
    V = tmax * P                               # gather num_idxs per position

    # wrapped int16 index layout for dma_gather: index i -> partition i%16,
    # col i//16, replicated across the 8 groups of 16 partitions.
    IW = CAP // 16
    idx_w = src_a.reshape(NB, 2, IW, 16).transpose(0, 1, 3, 2)  # [NB,2,16,IW]
    idx_w = np.ascontiguousarray(np.tile(idx_w, (1, 1, 8, 1)))  # [NB,2,128,IW]

    # col/w in per-tile scalar layout: [.., 128, 2*TH] where slot (h*TH+t)
    # on partition p = edge t*128+p of half h.
    colP = col_a.reshape(NB, 2, TH, P).transpose(3, 0, 1, 2).reshape(P, NB * 2 * TH)
    wfP = w_a.reshape(NB, 2, TH, P).transpose(3, 0, 1, 2).reshape(P, NB * 2 * TH)
    colP = np.ascontiguousarray(colP)
    wfP = np.ascontiguousarray(wfP)

    return dict(TH=TH, CAP=CAP, V=V, dinvP=dinvP, diagwP=diagwP,
                idx_w=idx_w, colP=colP, wfP=wfP)


def _host_golden(x, W1, b1, W2, b2, pp, out_dtype=np.float32, quant=True):
    """Numpy re-implementation of the device algorithm (same tiling, same
    bf16 quantization points). For validating the scheme off-device."""
    bf = (lambda a: a.astype(_BF16).astype(np.float32)) if quant else (lambda a: a)
    TH, CAP, V = pp["TH"], pp["CAP"], pp["V"]
    NB = NPAD // P

    dinv = pp["dinvP"].T.reshape(-1)
    diagw = pp["diagwP"].T.reshape(-1)

    xp = np.zeros((NPAD, FIN), np.float32)
    xp[:N] = x
    h1 = bf(xp) @ bf(W1)                     # bf16 inputs, f32 accum
    hs1 = bf(h1 * dinv[:, None])             # stored bf16

    idx_w = pp["idx_w"]
    colP = pp["colP"]
    wfP = np.asarray(pp["wfP"].astype(_BF16), dtype=np.float32)

    def agg(hs, F):
        out = np.zeros((NPAD, F), np.float32)
        for nb in range(NB):
            b = nb % BPC
            acc = (bf(diagw[nb * P:(nb + 1) * P])[:, None]
                   * hs[nb * P:(nb + 1) * P, :F])
            for hh in range(2):
                iw = idx_w[nb, hh, :16, :]                      # [16, IW]
                flat = iw.T.reshape(-1)[:CAP].astype(np.int64)  # unwrap
                base = 0 if hh == 0 else HALF
                for t in range(V[b, hh] // P):
                    oh = np.zeros((P, P), np.float32)
                    c = colP[:, (nb * 2 + hh) * TH + t]
                    wv = wfP[:, (nb * 2 + hh) * TH + t]
                    oh[np.arange(P), c.astype(np.int64)] = wv
                    msgs = hs[base + flat[t * P:(t + 1) * P]]
                    acc = acc + oh.T @ msgs
            out[nb * P:(nb + 1) * P] = acc
        return out

    out1 = agg(hs1, H) * dinv[:, None] + b1[None, :]
    out1 = np.maximum(out1, 0.0)

    h2 = bf(out1) @ bf(W2)
    hs2 = bf(h2 * dinv[:, None])
    out2 = agg(hs2, F2) * dinv[:, None] + b2[None, :]
    return out2[:N].astype(out_dtype)


# ---------------------------------------------------------------------------
# Bass device kernel
# ---------------------------------------------------------------------------

_NC_CACHE = {}


def _build_nc(TH, Vkey):
    import concourse.bass as bass  # noqa: F401
    import concourse.mybir as mybir
    import concourse.tile as tile
    from concourse import bacc
    from concourse.library_config import mlp

    DT = mybir.dt.bfloat16
    F32 = mybir.dt.float32
    I16 = mybir.dt.int16
    AL = mybir.AluOpType
    AF = mybir.ActivationFunctionType

    V = np.array(Vkey, dtype=np.int64).reshape(BPC, 2)
    CAP = TH * P
    IW = CAP // 16
    NB = NPAD // P           # 392

    nc = bacc.Bacc("TRN2", target_bir_lowering=False, debug=True,
                   num_devices=NCORES, num_swdge_queues=4)
    xt3_d = nc.dram_tensor("xt3", [2, P, NPAD], DT, kind="ExternalInput")
    w1_d = nc.dram_tensor("w1c", [2, P, H], DT, kind="ExternalInput")
    w2_d = nc.dram_tensor("w2c", [2, P, F2], DT, kind="ExternalInput")
    b1_d = nc.dram_tensor("b1f", [P, H], F32, kind="ExternalInput")
    b2_d = nc.dram_tensor("b2f", [P, F2], F32, kind="ExternalInput")
    iota_d = nc.dram_tensor("iota", [P, P], DT, kind="ExternalInput")
    pidx_d = nc.dram_tensor("pidxf", [P, 1], F32, kind="ExternalInput")
    dinv_d = nc.dram_tensor("dinvP", [P, NB], F32, kind="ExternalInput")
    dinvl_d = nc.dram_tensor("dinvlP", [P, BPC], F32, kind="ExternalInput")
    diagwl_d = nc.dram_tensor("diagwlP", [P, BPC], F32, kind="ExternalInput")
    idx_d = nc.dram_tensor("idxP", [P, BPC * 2 * IW], I16, kind="ExternalInput")
    col_d = nc.dram_tensor("colP", [P, BPC * 2 * TH], F32, kind="ExternalInput")
    wf_d = nc.dram_tensor("wfP", [P, BPC * 2 * TH], F32, kind="ExternalInput")
    out_d = nc.dram_tensor("out2", [SHARD, F2], F32, kind="ExternalOutput")

    with tile.TileContext(nc) as tc:
        with (
            tc.tile_pool(name="dram", bufs=1, space="DRAM") as dpool,
            tc.tile_pool(name="const", bufs=1) as cpool,
            tc.tile_pool(name="xs", bufs=4) as xpool,
            tc.tile_pool(name="hst", bufs=4) as hpool,
            tc.tile_pool(name="msg", bufs=5) as mpool,
            tc.tile_pool(name="oh", bufs=24) as ohpool,
            tc.tile_pool(name="post", bufs=3) as tpool,
            tc.tile_pool(name="ph1", bufs=3, space="PSUM") as ph1p,
            tc.tile_pool(name="pagg", bufs=3, space="PSUM") as paggp,
            tc.tile_pool(name="pc", bufs=2, space="PSUM") as pcp,
        ):
            hs1_tab = dpool.tile([NPAD, H], DT)
            h2in_dram = dpool.tile([SHARD, H], DT)
            hs2_shard = dpool.tile([SHARD, F2], DT)
            hs2_full = dpool.tile([NPAD, F2], DT, addr_space="Shared")

            nc.gpsimd.load_library(mlp)

            # ---- constants ----
            w1_sb = cpool.tile([P, 2 * H], DT)
            nc.sync.dma_start(out=w1_sb[:, 0:H], in_=w1_d[0])
            nc.sync.dma_start(out=w1_sb[:, H:2 * H], in_=w1_d[1])
            w2_sb = cpool.tile([P, 2 * F2], DT)
            nc.sync.dma_start(out=w2_sb[:, 0:F2], in_=w2_d[0])
            nc.sync.dma_start(out=w2_sb[:, F2:2 * F2], in_=w2_d[1])
            b1_sb = cpool.tile([P, H], F32)
            nc.sync.dma_start(out=b1_sb[:], in_=b1_d[:])
            b2_sb = cpool.tile([P, F2], F32)
            nc.sync.dma_start(out=b2_sb[:], in_=b2_d[:])
            iota_sb = cpool.tile([P, P], DT)
            nc.sync.dma_start(out=iota_sb[:], in_=iota_d[:])
            pidx_sb = cpool.tile([P, 1], F32)
            nc.sync.dma_start(out=pidx_sb[:], in_=pidx_d[:])
            idx_sb = cpool.tile([P, BPC * 2 * IW], I16)
            nc.sync.dma_start(out=idx_sb[:], in_=idx_d[:])
            col_sb = cpool.tile([P, BPC * 2 * TH], F32)
            nc.sync.dma_start(out=col_sb[:], in_=col_d[:])
            wf_sb = cpool.tile([P, BPC * 2 * TH], F32)
            nc.sync.dma_start(out=wf_sb[:], in_=wf_d[:])
            dinv_sb = cpool.tile([P, NB], F32)
            nc.sync.dma_start(out=dinv_sb[:], in_=dinv_d[:])
            dinvl_sb = cpool.tile([P, BPC], F32)
            nc.sync.dma_start(out=dinvl_sb[:], in_=dinvl_d[:])
            diagwl_sb = cpool.tile([P, BPC], F32)
            nc.sync.dma_start(out=diagwl_sb[:], in_=diagwl_d[:])

            # ---- phase A: h1 = x @ W1 (all nodes), hs1 = h1 * dinv ----
            # two blocks share one PSUM bank ([P, 512] f32); paired writes
            for s in range(NPAD // 512):
                xa = xpool.tile([P, 512], DT, tag="xa")
                xb = xpool.tile([P, 512], DT, tag="xb")
                nc.sync.dma_start(out=xa[:], in_=xt3_d[0][:, s * 512:(s + 1) * 512])
                nc.sync.dma_start(out=xb[:], in_=xt3_d[1][:, s * 512:(s + 1) * 512])
                for hq in range(2):
                    nb0 = s * 4 + hq * 2
                    ph = ph1p.tile([P, 2 * H], F32)
                    for j in range(2):
                        q = hq * 2 + j
                        nc.tensor.matmul(ph[:, j * H:(j + 1) * H],
                                         lhsT=xa[:, q * P:(q + 1) * P],
                                         rhs=w1_sb[:, 0:H], start=True, stop=False)
                        nc.tensor.matmul(ph[:, j * H:(j + 1) * H],
                                         lhsT=xb[:, q * P:(q + 1) * P],
                                         rhs=w1_sb[:, H:2 * H], start=False,
                                         stop=True)
                    hst = hpool.tile([P, 2, H], DT, tag="hst")
                    for j in range(2):
                        nb = nb0 + j
                        if j == 0:
                            nc.scalar.activation(hst[:, j, :], ph[:, j * H:(j + 1) * H],
                                                 AF.Copy, scale=dinv_sb[:, nb:nb + 1])
                        else:
                            nc.vector.tensor_scalar(hst[:, j, :], ph[:, j * H:(j + 1) * H],
                                                    dinv_sb[:, nb:nb + 1], None,
                                                    AL.mult)
                    nc.sync.dma_start(
                        out=hs1_tab[nb0 * P:(nb0 + 2) * P, :].rearrange(
                            "(i p) f -> p i f", p=P),
                        in_=hst[:])

            pid_sc = nc.scalar.partition_id()

            def aggregate(b, tab, tab_own, F, psum_pool, psum_tag):
                """Edge aggregation for dest block b from table `tab` (+ the
                diagonal term from the core's own block). Returns PSUM tile."""
                msgs = []
                for hh in range(2):
                    nt = int(V[b, hh]) // P
                    m = mpool.tile([P, TH, F], DT, tag=f"msg{hh}")
                    src = tab[0:HALF, :] if hh == 0 else tab[HALF:NPAD, :]
                    nc.gpsimd.dma_gather(
                        m[:, 0:nt, :], src,
                        idx_sb[:, (b * 2 + hh) * IW:(b * 2 + hh) * IW + nt * 8],
                        int(V[b, hh]), int(V[b, hh]), F, single_packet=False,
                        queue_num=(b * 2 + hh) % 4)
                    msgs.append(m)
                own = ohpool.tile([P, F], DT, tag="own")
                if tab_own is None:
                    nc.scalar.dma_start(
                        out=own[:],
                        in_=tab[bass.ds((pid_sc * BPC + b) * P, P), 0:F])
                else:
                    nc.scalar.dma_start(
                        out=own[:], in_=tab_own[b * P:(b + 1) * P, 0:F])
                dg = ohpool.tile([P, P], DT, tag="oh")
                nc.vector.tensor_scalar(dg[:], iota_sb[:], pidx_sb[:, 0:1],
                                        diagwl_sb[:, b:b + 1], AL.is_equal,
                                        AL.mult)
                pagg = psum_pool.tile([P, F], F32, tag=psum_tag)
                t = 0
                for hh in range(2):
                    for tt in range(int(V[b, hh]) // P):
                        oh = ohpool.tile([P, P], DT, tag="oh")
                        sc = (b * 2 + hh) * TH + tt
                        nc.vector.tensor_scalar(oh[:], iota_sb[:],
                                                col_sb[:, sc:sc + 1],
                                                wf_sb[:, sc:sc + 1],
                                                AL.is_equal, AL.mult)
                        nc.tensor.matmul(pagg[:], lhsT=oh[:],
                                         rhs=msgs[hh][:, tt, :],
                                         start=(t == 0), stop=False)
                        t += 1
                nc.tensor.matmul(pagg[:], lhsT=dg[:], rhs=own[:],
                                 start=False, stop=True)
                return pagg

            # ---- phase B: layer-1 aggregation per dest block, with the
            #      layer-2 projection (relu_out @ W2) interleaved ----
            for b in range(BPC):
                pagg = aggregate(b, hs1_tab, None, H, paggp, "")
                t1 = tpool.tile([P, H], F32, tag="t1")
                nc.vector.tensor_scalar(t1[:], pagg[:], dinvl_sb[:, b:b + 1], None,
                                        AL.mult)
                t2 = tpool.tile([P, H], F32, tag="t2")
                nc.vector.tensor_tensor(t2[:], t1[:], b1_sb[:], AL.add)
                rl = hpool.tile([P, H], DT, tag="rl")
                nc.scalar.activation(rl[:], t2[:], AF.Relu)
                nc.scalar.dma_start(out=h2in_dram[b * P:(b + 1) * P, :], in_=rl[:])

                # layer-2 projection for this block
                ph2 = pcp.tile([P, F2], F32, tag="pc")
                for c2 in range(2):
                    at = ohpool.tile([P, P], DT, tag="at")
                    nc.scalar.dma_start(
                        out=at[:],
                        in_=h2in_dram[b * P:(b + 1) * P, c2 * P:(c2 + 1) * P],
                        transpose=True)
                    nc.tensor.matmul(ph2[:], lhsT=at[:],
                                     rhs=w2_sb[:, c2 * F2:(c2 + 1) * F2],
                                     start=(c2 == 0), stop=(c2 == 1))
                hsb = hpool.tile([P, F2], DT, tag="hsb")
                nc.scalar.activation(hsb[:], ph2[:], AF.Copy,
                                     scale=dinvl_sb[:, b:b + 1])
                nc.scalar.dma_start(out=hs2_shard[b * P:(b + 1) * P, :], in_=hsb[:])

            # ---- phase D: exchange hs2 shards ----
            nc.gpsimd.collective_compute(
                "AllGather", AL.bypass,
                replica_groups=[list(range(NCORES))],
                ins=[hs2_shard[:]],
                outs=[hs2_full[:]],
            )

            # ---- phase E: layer-2 aggregation per dest block ----
            for b in range(BPC):
                pagg2 = aggregate(b, hs2_full, hs2_shard, F2, pcp, "pc")
                o1 = tpool.tile([P, F2], F32, tag="o1")
                nc.vector.tensor_scalar(o1[:], pagg2[:], dinvl_sb[:, b:b + 1], None,
                                        AL.mult)
                o2 = tpool.tile([P, F2], F32, tag="o2")
                nc.vector.tensor_tensor(o2[:], o1[:], b2_sb[:], AL.add)
                nc.sync.dma_start(out=out_d[b * P:(b + 1) * P, :], in_=o2[:])

    nc.compile()
    return nc


def _make_inputs(x, W1, b1, W2, b2, pp):
    """Per-core input maps."""
    TH = pp["TH"]
    IW = (TH * P) // 16
    NB = NPAD // P

    xp = np.zeros((NPAD, FIN), np.float32)
    xp[:N] = x
    xt3 = np.ascontiguousarray(
        xp.T.reshape(2, P, NPAD).astype(_BF16))
    w1c = np.ascontiguousarray(W1.reshape(2, P, H).astype(_BF16))
    w2c = np.ascontiguousarray(W2.reshape(2, P, F2).astype(_BF16))
    b1f = np.ascontiguousarray(np.tile(b1[None, :], (P, 1)).astype(np.float32))
    b2f = np.ascontiguousarray(np.tile(b2[None, :], (P, 1)).astype(np.float32))
    iota = np.tile(np.arange(P, dtype=np.float32)[None, :], (P, 1)).astype(_BF16)
    pidxf = np.arange(P, dtype=np.float32)[:, None].copy()

    dinvP = pp["dinvP"]
    diagwP = pp["diagwP"]
    idx_w = pp["idx_w"]        # [NB, 2, 128, IW]
    colP = pp["colP"]          # [128, NB*2*TH]
    wfP = pp["wfP"]

    in_maps = []
    for c in range(NCORES):
        b0 = c * BPC
        idxP = np.ascontiguousarray(
            idx_w[b0:b0 + BPC].transpose(2, 0, 1, 3).reshape(P, BPC * 2 * IW))
        in_maps.append({
            "xt3": xt3,
            "w1c": w1c,
            "w2c": w2c,
            "b1f": b1f,
            "b2f": b2f,
            "iota": iota,
            "pidxf": pidxf,
            "dinvP": dinvP,
            "dinvlP": np.ascontiguousarray(dinvP[:, b0:b0 + BPC]),
            "diagwlP": np.ascontiguousarray(diagwP[:, b0:b0 + BPC]),
            "idxP": idxP,
            "colP": np.ascontiguousarray(colP[:, b0 * 2 * TH:(b0 + BPC) * 2 * TH]),
            "wfP": np.ascontiguousarray(wfP[:, b0 * 2 * TH:(b0 + BPC) * 2 * TH]),
        })
    return in_maps


def kernel(x, edge_index, edge_weight, W1, b1, W2, b2, _trace=False):
    from concourse.bass_utils import run_bass_kernel_spmd

    x = np.asarray(x, dtype=np.float32)
    W1 = np.asarray(W1, dtype=np.float32)
    b1 = np.asarray(b1, dtype=np.float32)
    W2 = np.asarray(W2, dtype=np.float32)
    b2 = np.asarray(b2, dtype=np.float32)

    pp = _preprocess(np.asarray(edge_index), np.asarray(edge_weight))
    key = (pp["TH"], tuple(pp["V"].reshape(-1).tolist()))
    if key not in _NC_CACHE:
        _NC_CACHE[key] = _build_nc(*key)
    nc = _NC_CACHE[key]

    in_maps = _make_inputs(x, W1, b1, W2, b2, pp)
    res = run_bass_kernel_spmd(nc, in_maps, list(range(NCORES)), trace=_trace)
    out = np.concatenate([res.results[c]["out2"] for c in range(NCORES)], axis=0)
    if _trace:
        kernel._last_result = res
    return np.ascontiguousarray(out[:N])


# revision 13
# speedup vs baseline: 1.4501x; 1.0040x over previous
"""GCN encoder (2-layer) Bass kernel for Trainium2, 8 NeuronCores.

Strategy (graph/data parallel, per sharding hint):
  - Nodes padded to NPAD=50176 and sharded by contiguous range: core c owns
    destination nodes [c*6272, (c+1)*6272) = 49 blocks of 128.
  - Edges (incl. self-loops) are bucketed by destination block and by source
    half (dma_gather indices are int16, so the feature table is gathered in
    two halves of 25088 rows each), sorted by source within each bucket for
    HBM page locality.  Edges with src==dst ("diagonal" edges, incl. all
    self-loops) are pulled out of the buckets and handled by one per-block
    diagonal matmul instead (no gather needed).  Each (block,half) bucket is
    padded to V = max-over-cores ceil128(count) so all 8 cores run one
    identical SPMD program with per-gather-exact descriptor counts (the Q7
    descriptor-generation on the GpSimd engine is the kernel bottleneck).
  - Per layer: h = x @ W (dense matmul, PSUM f32), table hs = h * dinv[src]
    stored in HBM (bf16); per destination block, edge messages are fetched
    with dma_gather (128 edges/tile, edge-major) and segment-summed on the
    TensorEngine via one-hot matmuls: onehot[k,d] = w[k] * (col[k]==d), so
    PSUM[d,f] += sum_k w[k]*hs[src_k][f]. Post: * dinv[dest] + bias (+relu).
  - The layer-2 projection (relu_out @ W2) is interleaved into the layer-1
    block loop; hs2 shards are then exchanged with an AllGather collective.
  - dinv = rsqrt(deg) is precomputed on the host (graph-structure preproc).

kernel(**inputs) takes the FULL inputs and returns the FULL [50000,128] f32
output; all sharding/gather happens inside.
"""

import sys

sys.path.insert(0, "/opt/trn_rl_repo")

import numpy as np
import ml_dtypes

P = 128
NCORES = 8
BPC = 49                 # dest blocks per core
SHARD = BPC * P          # 6272
NPAD = NCORES * SHARD    # 50176
HALF = NPAD // 2         # 25088
N = 50000
FIN = 256
H = 256                  # layer-1 output width
F2 = 128                 # layer-2 output width
DUMMY_SRC = N + 8        # a zero (pad) node, used as src for pad edges

_BF16 = ml_dtypes.bfloat16


def _preprocess(edge_index, edge_weight):
    """Build all per-core device input arrays from the edge list."""
    row = np.asarray(edge_index[0], dtype=np.int64)
    col = np.asarray(edge_index[1], dtype=np.int64)
    w = np.asarray(edge_weight, dtype=np.float32)

    loop = np.arange(N, dtype=np.int64)
    rows = np.concatenate([row, loop])
    cols = np.concatenate([col, loop])
    ws = np.concatenate([w, np.ones(N, np.float32)])

    # ---- dinv on host (graph-structure preprocessing) ----
    deg = np.zeros(NPAD, np.float32)
    np.add.at(deg, cols, ws)
    deg = deg + (deg == 0)
    dinv = np.sqrt(1.0 / deg).astype(np.float32)
    # partition-major [128, 392]: dinvP[p, nb] = dinv[nb*128+p]
    dinvP = np.ascontiguousarray(dinv.reshape(NPAD // P, P).T)

    # ---- diagonal (src==dst) edges -> per-node diag weight ----
    isdiag = rows == cols
    diagw = np.zeros(NPAD, np.float32)
    np.add.at(diagw, cols[isdiag], ws[isdiag])
    diagwP = np.ascontiguousarray(diagw.reshape(NPAD // P, P).T)

    rows = rows[~isdiag]
    cols = cols[~isdiag]
    ws = ws[~isdiag]
    EE = rows.shape[0]

    # ---- edge streams per (block, half), sorted by src within bucket ----
    blk = cols // P                      # 0..391
    half = (rows >= HALF).astype(np.int64)
    key = blk * 2 + half
    cnt = np.bincount(key, minlength=(NPAD // P) * 2)
    TH = int(-(-cnt.max() // P))         # tiles per half (max over all)
    CAP = TH * P
    NB = NPAD // P                       # 392 blocks

    src_a = np.full((NB, 2, CAP), DUMMY_SRC % HALF, np.int16)
    col_a = np.zeros((NB, 2, CAP), np.float32)
    w_a = np.zeros((NB, 2, CAP), np.float32)

    # sort by (bucket, src) so gathered rows are ascending within a bucket
    order2 = np.lexsort((rows, key))
    cs2 = np.zeros(NB * 2 + 1, np.int64)
    np.cumsum(cnt, out=cs2[1:])
    pos = np.arange(EE) - cs2[key[order2]]
    kb = key[order2] // 2
    kh = key[order2] % 2
    src_sorted = rows[order2]
    src_rel = np.where(kh == 1, src_sorted - HALF, src_sorted).astype(np.int16)
    src_a[kb, kh, pos] = src_rel
    col_a[kb, kh, pos] = (cols[order2] - kb * P).astype(np.float32)
    w_a[kb, kh, pos] = ws[order2]

    # per-(block,half) gather length: max over the 8 cores at the same block
    # position of ceil128(count); identical across cores -> static program.
    cnt2 = cnt.reshape(NCORES, BPC, 2)
    tcnt = -(-cnt2 // P)                       # tiles, [8, 49, 2]
    tmax = tcnt.max(axis=0)                    # [49, 2]
    V = tmax * P                               # gather num_idxs per position

    # wrapped int16 index layout for dma_gather: index i -> partition i%16,
    # col i//16, replicated across the 8 groups of 16 partitions.
    IW = CAP // 16
    idx_w = src_a.reshape(NB, 2, IW, 16).transpose(0, 1, 3, 2)  # [NB,2,16,IW]
    idx_w = np.ascontiguousarray(np.tile(idx_w, (1, 1, 8, 1)))  # [NB,2,128,IW]

    # col/w in per-tile scalar layout: [.., 128, 2*TH] where slot (h*TH+t)
    # on partition p = edge t*128+p of half h.
    colP = col_a.reshape(NB, 2, TH, P).transpose(3, 0, 1, 2).reshape(P, NB * 2 * TH)
    wfP = w_a.reshape(NB, 2, TH, P).transpose(3, 0, 1, 2).reshape(P, NB * 2 * TH)
    colP = np.ascontiguousarray(colP)
    wfP = np.ascontiguousarray(wfP)

    return dict(TH=TH, CAP=CAP, V=V, dinvP=dinvP, diagwP=diagwP,
                idx_w=idx_w, colP=colP, wfP=wfP)


def _host_golden(x, W1, b1, W2, b2, pp, out_dtype=np.float32, quant=True):
    """Numpy re-implementation of the device algorithm (same tiling, same
    bf16 quantization points). For validating the scheme off-device."""
    bf = (lambda a: a.astype(_BF16).astype(np.float32)) if quant else (lambda a: a)
    TH, CAP, V = pp["TH"], pp["CAP"], pp["V"]
    NB = NPAD // P

    dinv = pp["dinvP"].T.reshape(-1)
    diagw = pp["diagwP"].T.reshape(-1)

    xp = np.zeros((NPAD, FIN), np.float32)
    xp[:N] = x
    h1 = bf(xp) @ bf(W1)                     # bf16 inputs, f32 accum
    hs1 = bf(h1 * dinv[:, None])             # stored bf16

    idx_w = pp["idx_w"]
    colP = pp["colP"]
    wfP = np.asarray(pp["wfP"].astype(_BF16), dtype=np.float32)

    def agg(hs, F):
        out = np.zeros((NPAD, F), np.float32)
        for nb in range(NB):
            b = nb % BPC
            acc = (bf(diagw[nb * P:(nb + 1) * P])[:, None]
                   * hs[nb * P:(nb + 1) * P, :F])
            for hh in range(2):
                iw = idx_w[nb, hh, :16, :]                      # [16, IW]
                flat = iw.T.reshape(-1)[:CAP].astype(np.int64)  # unwrap
                base = 0 if hh == 0 else HALF
                for t in range(V[b, hh] // P):
                    oh = np.zeros((P, P), np.float32)
                    c = colP[:, (nb * 2 + hh) * TH + t]
                    wv = wfP[:, (nb * 2 + hh) * TH + t]
                    oh[np.arange(P), c.astype(np.int64)] = wv
                    msgs = hs[base + flat[t * P:(t + 1) * P]]
                    acc = acc + oh.T @ msgs
            out[nb * P:(nb + 1) * P] = acc
        return out

    out1 = agg(hs1, H) * dinv[:, None] + b1[None, :]
    out1 = np.maximum(out1, 0.0)

    h2 = bf(out1) @ bf(W2)
    hs2 = bf(h2 * dinv[:, None])
    out2 = agg(hs2, F2) * dinv[:, None] + b2[None, :]
    return out2[:N].astype(out_dtype)


# ---------------------------------------------------------------------------
# Bass device kernel
# ---------------------------------------------------------------------------

_NC_CACHE = {}


def _build_nc(TH, Vkey):
    import concourse.bass as bass  # noqa: F401
    import concourse.mybir as mybir
    import concourse.tile as tile
    from concourse import bacc
    from concourse.library_config import mlp

    DT = mybir.dt.bfloat16
    F32 = mybir.dt.float32
    I16 = mybir.dt.int16
    AL = mybir.AluOpType
    AF = mybir.ActivationFunctionType

    V = np.array(Vkey, dtype=np.int64).reshape(BPC, 2)
    CAP = TH * P
    IW = CAP // 16
    NB = NPAD // P           # 392

    nc = bacc.Bacc("TRN2", target_bir_lowering=False, debug=True,
                   num_devices=NCORES, num_swdge_queues=4)
    xt3_d = nc.dram_tensor("xt3", [2, P, NPAD], DT, kind="ExternalInput")
    w1_d = nc.dram_tensor("w1c", [2, P, H], DT, kind="ExternalInput")
    w2_d = nc.dram_tensor("w2c", [2, P, F2], DT, kind="ExternalInput")
    b1_d = nc.dram_tensor("b1f", [P, H], F32, kind="ExternalInput")
    b2_d = nc.dram_tensor("b2f", [P, F2], F32, kind="ExternalInput")
    iota_d = nc.dram_tensor("iota", [P, P], DT, kind="ExternalInput")
    pidx_d = nc.dram_tensor("pidxf", [P, 1], F32, kind="ExternalInput")
    dinv_d = nc.dram_tensor("dinvP", [P, NB], F32, kind="ExternalInput")
    dinvl_d = nc.dram_tensor("dinvlP", [P, BPC], F32, kind="ExternalInput")
    diagwl_d = nc.dram_tensor("diagwlP", [P, BPC], F32, kind="ExternalInput")
    idx_d = nc.dram_tensor("idxP", [P, BPC * 2 * IW], I16, kind="ExternalInput")
    col_d = nc.dram_tensor("colP", [P, BPC * 2 * TH], F32, kind="ExternalInput")
    wf_d = nc.dram_tensor("wfP", [P, BPC * 2 * TH], F32, kind="ExternalInput")
    out_d = nc.dram_tensor("out2", [SHARD, F2], F32, kind="ExternalOutput")

    with tile.TileContext(nc) as tc:
        with (
            tc.tile_pool(name="dram", bufs=1, space="DRAM") as dpool,
            tc.tile_pool(name="const", bufs=1) as cpool,
            tc.tile_pool(name="xs", bufs=3) as xpool,
            tc.tile_pool(name="hst", bufs=3) as hpool,
            tc.tile_pool(name="msg", bufs=6) as mpool,
            tc.tile_pool(name="oh", bufs=16) as ohpool,
            tc.tile_pool(name="post", bufs=3) as tpool,
            tc.tile_pool(name="ph1", bufs=3, space="PSUM") as ph1p,
            tc.tile_pool(name="pagg", bufs=3, space="PSUM") as paggp,
            tc.tile_pool(name="pc", bufs=2, space="PSUM") as pcp,
        ):
            hs1_tab = dpool.tile([NPAD, H], DT)
            h2in_dram = dpool.tile([SHARD, H], DT)
            hs2_shard = dpool.tile([SHARD, F2], DT)
            hs2_full = dpool.tile([NPAD, F2], DT, addr_space="Shared")

            nc.gpsimd.load_library(mlp)

            # ---- constants ----
            w1_sb = cpool.tile([P, 2 * H], DT)
            nc.sync.dma_start(out=w1_sb[:, 0:H], in_=w1_d[0])
            nc.sync.dma_start(out=w1_sb[:, H:2 * H], in_=w1_d[1])
            w2_sb = cpool.tile([P, 2 * F2], DT)
            nc.sync.dma_start(out=w2_sb[:, 0:F2], in_=w2_d[0])
            nc.sync.dma_start(out=w2_sb[:, F2:2 * F2], in_=w2_d[1])
            b1_sb = cpool.tile([P, H], F32)
            nc.sync.dma_start(out=b1_sb[:], in_=b1_d[:])
            b2_sb = cpool.tile([P, F2], F32)
            nc.sync.dma_start(out=b2_sb[:], in_=b2_d[:])
            iota_sb = cpool.tile([P, P], DT)
            nc.sync.dma_start(out=iota_sb[:], in_=iota_d[:])
            pidx_sb = cpool.tile([P, 1], F32)
            nc.sync.dma_start(out=pidx_sb[:], in_=pidx_d[:])
            idx_sb = cpool.tile([P, BPC * 2 * IW], I16)
            nc.sync.dma_start(out=idx_sb[:], in_=idx_d[:])
            col_sb = cpool.tile([P, BPC * 2 * TH], F32)
            nc.sync.dma_start(out=col_sb[:], in_=col_d[:])
            wf_sb = cpool.tile([P, BPC * 2 * TH], F32)
            nc.sync.dma_start(out=wf_sb[:], in_=wf_d[:])
            dinv_sb = cpool.tile([P, NB], F32)
            nc.sync.dma_start(out=dinv_sb[:], in_=dinv_d[:])
            dinvl_sb = cpool.tile([P, BPC], F32)
            nc.sync.dma_start(out=dinvl_sb[:], in_=dinvl_d[:])
            diagwl_sb = cpool.tile([P, BPC], F32)
            nc.sync.dma_start(out=diagwl_sb[:], in_=diagwl_d[:])

            # ---- phase A: h1 = x @ W1 (all nodes), hs1 = h1 * dinv ----
            # two blocks share one PSUM bank ([P, 512] f32); paired writes
            for s in range(NPAD // 512):
                xa = xpool.tile([P, 512], DT, tag="xa")
                xb = xpool.tile([P, 512], DT, tag="xb")
                nc.sync.dma_start(out=xa[:], in_=xt3_d[0][:, s * 512:(s + 1) * 512])
                nc.sync.dma_start(out=xb[:], in_=xt3_d[1][:, s * 512:(s + 1) * 512])
                for hq in range(2):
                    nb0 = s * 4 + hq * 2
                    ph = ph1p.tile([P, 2 * H], F32)
                    for j in range(2):
                        q = hq * 2 + j
                        nc.tensor.matmul(ph[:, j * H:(j + 1) * H],
                                         lhsT=xa[:, q * P:(q + 1) * P],
                                         rhs=w1_sb[:, 0:H], start=True, stop=False)
                        nc.tensor.matmul(ph[:, j * H:(j + 1) * H],
                                         lhsT=xb[:, q * P:(q + 1) * P],
                                         rhs=w1_sb[:, H:2 * H], start=False,
                                         stop=True)
                    hst = hpool.tile([P, 2, H], DT, tag="hst")
                    for j in range(2):
                        nb = nb0 + j
                        if j == 0:
                            nc.scalar.activation(hst[:, j, :], ph[:, j * H:(j + 1) * H],
                                                 AF.Copy, scale=dinv_sb[:, nb:nb + 1])
                        else:
                            nc.vector.tensor_scalar(hst[:, j, :], ph[:, j * H:(j + 1) * H],
                                                    dinv_sb[:, nb:nb + 1], None,
                                                    AL.mult)
                    nc.sync.dma_start(
                        out=hs1_tab[nb0 * P:(nb0 + 2) * P, :].rearrange(
                            "(i p) f -> p i f", p=P),
                        in_=hst[:])

            pid_sc = nc.scalar.partition_id()

            def aggregate(b, tab, tab_own, F, psum_pool, psum_tag):
                """Edge aggregation for dest block b from table `tab` (+ the
                diagonal term from the core's own block). Returns PSUM tile."""
                msgs = []
                for hh in range(2):
                    nt = int(V[b, hh]) // P
                    m = mpool.tile([P, TH, F], DT, tag=f"msg{hh}")
                    src = tab[0:HALF, :] if hh == 0 else tab[HALF:NPAD, :]
                    nc.gpsimd.dma_gather(
                        m[:, 0:nt, :], src,
                        idx_sb[:, (b * 2 + hh) * IW:(b * 2 + hh) * IW + nt * 8],
                        int(V[b, hh]), int(V[b, hh]), F, single_packet=False,
                        queue_num=(b * 2 + hh) % 4)
                    msgs.append(m)
                own = ohpool.tile([P, F], DT, tag="own")
                if tab_own is None:
                    nc.scalar.dma_start(
                        out=own[:],
                        in_=tab[bass.ds((pid_sc * BPC + b) * P, P), 0:F])
                else:
                    nc.scalar.dma_start(
                        out=own[:], in_=tab_own[b * P:(b + 1) * P, 0:F])
                dg = ohpool.tile([P, P], DT, tag="oh")
                nc.vector.tensor_scalar(dg[:], iota_sb[:], pidx_sb[:, 0:1],
                                        diagwl_sb[:, b:b + 1], AL.is_equal,
                                        AL.mult)
                pagg = psum_pool.tile([P, F], F32, tag=psum_tag)
                t = 0
                for hh in range(2):
                    for tt in range(int(V[b, hh]) // P):
                        oh = ohpool.tile([P, P], DT, tag="oh")
                        sc = (b * 2 + hh) * TH + tt
                        nc.vector.tensor_scalar(oh[:], iota_sb[:],
                                                col_sb[:, sc:sc + 1],
                                                wf_sb[:, sc:sc + 1],
                                                AL.is_equal, AL.mult)
                        nc.tensor.matmul(pagg[:], lhsT=oh[:],
                                         rhs=msgs[hh][:, tt, :],
                                         start=(t == 0), stop=False)
                        t += 1
                nc.tensor.matmul(pagg[:], lhsT=dg[:], rhs=own[:],
                                 start=False, stop=True)
                return pagg

            # ---- phase B: layer-1 aggregation per dest block, with the
            #      layer-2 projection (relu_out @ W2) interleaved ----
            for b in range(BPC):
                pagg = aggregate(b, hs1_tab, None, H, paggp, "")
                t1 = tpool.tile([P, H], F32, tag="t1")
                nc.vector.tensor_scalar(t1[:], pagg[:], dinvl_sb[:, b:b + 1], None,
                                        AL.mult)
                t2 = tpool.tile([P, H], F32, tag="t2")
                nc.vector.tensor_tensor(t2[:], t1[:], b1_sb[:], AL.add)
                rl = hpool.tile([P, H], DT, tag="rl")
                nc.scalar.activation(rl[:], t2[:], AF.Relu)
                nc.scalar.dma_start(out=h2in_dram[b * P:(b + 1) * P, :], in_=rl[:])

                # layer-2 projection for this block
                ph2 = pcp.tile([P, F2], F32, tag="pc")
                for c2 in range(2):
                    at = ohpool.tile([P, P], DT, tag="at")
                    nc.scalar.dma_start(
                        out=at[:],
                        in_=h2in_dram[b * P:(b + 1) * P, c2 * P:(c2 + 1) * P],
                        transpose=True)
                    nc.tensor.matmul(ph2[:], lhsT=at[:],
                                     rhs=w2_sb[:, c2 * F2:(c2 + 1) * F2],
                                     start=(c2 == 0), stop=(c2 == 1))
                hsb = hpool.tile([P, F2], DT, tag="hsb")
                nc.scalar.activation(hsb[:], ph2[:], AF.Copy,
                                     scale=dinvl_sb[:, b:b + 1])
                nc.scalar.dma_start(out=hs2_shard[b * P:(b + 1) * P, :], in_=hsb[:])

            # ---- phase D: exchange hs2 shards ----
            nc.gpsimd.collective_compute(
                "AllGather", AL.bypass,
                replica_groups=[list(range(NCORES))],
                ins=[hs2_shard[:]],
                outs=[hs2_full[:]],
            )

            # ---- phase E: layer-2 aggregation per dest block ----
            for b in range(BPC):
                pagg2 = aggregate(b, hs2_full, hs2_shard, F2, pcp, "pc")
                o1 = tpool.tile([P, F2], F32, tag="o1")
                nc.vector.tensor_scalar(o1[:], pagg2[:], dinvl_sb[:, b:b + 1], None,
                                        AL.mult)
                o2 = tpool.tile([P, F2], F32, tag="o2")
                nc.vector.tensor_tensor(o2[:], o1[:], b2_sb[:], AL.add)
                nc.sync.dma_start(out=out_d[b * P:(b + 1) * P, :], in_=o2[:])

    nc.compile()
    return nc


def _make_inputs(x, W1, b1, W2, b2, pp):
    """Per-core input maps."""
    TH = pp["TH"]
    IW = (TH * P) // 16
    NB = NPAD // P

    xp = np.zeros((NPAD, FIN), np.float32)
    xp[:N] = x
    xt3 = np.ascontiguousarray(
        xp.T.reshape(2, P, NPAD).astype(_BF16))
    w1c = np.ascontiguousarray(W1.reshape(2, P, H).astype(_BF16))
    w2c = np.ascontiguousarray(W2.reshape(2, P, F2).astype(_BF16))
    b1f = np.ascontiguousarray(np.tile(b1[None, :], (P, 1)).astype(np.float32))
    b2f = np.ascontiguousarray(np.tile(b2[None, :], (P, 1)).astype(np.float32))
    iota = np.tile(np.arange(P, dtype=np.float32)[None, :], (P, 1)).astype(_BF16)
    pidxf = np.arange(P, dtype=np.float32)[:, None].copy()

    dinvP = pp["dinvP"]
    diagwP = pp["diagwP"]
    idx_w = pp["idx_w"]        # [NB, 2, 128, IW]
    colP = pp["colP"]          # [128, NB*2*TH]
    wfP = pp["wfP"]

    in_maps = []
    for c in range(NCORES):
        b0 = c * BPC
        idxP = np.ascontiguousarray(
            idx_w[b0:b0 + BPC].transpose(2, 0, 1, 3).reshape(P, BPC * 2 * IW))
        in_maps.append({
            "xt3": xt3,
            "w1c": w1c,
            "w2c": w2c,
            "b1f": b1f,
            "b2f": b2f,
            "iota": iota,
            "pidxf": pidxf,
            "dinvP": dinvP,
            "dinvlP": np.ascontiguousarray(dinvP[:, b0:b0 + BPC]),
            "diagwlP": np.ascontiguousarray(diagwP[:, b0:b0 + BPC]),
            "idxP": idxP,
            "colP": np.ascontiguousarray(colP[:, b0 * 2 * TH:(b0 + BPC) * 2 * TH]),
            "wfP": np.ascontiguousarray(wfP[:, b0 * 2 * TH:(b0 + BPC) * 2 * TH]),
        })
    return in_maps


def kernel(x, edge_index, edge_weight, W1, b1, W2, b2, _trace=False):
    from concourse.bass_utils import run_bass_kernel_spmd

    x = np.asarray(x, dtype=np.float32)
    W1 = np.asarray(W1, dtype=np.float32)
    b1 = np.asarray(b1, dtype=np.float32)
    W2 = np.asarray(W2, dtype=np.float32)
    b2 = np.asarray(b2, dtype=np.float32)

    pp = _preprocess(np.asarray(edge_index), np.asarray(edge_weight))
    key = (pp["TH"], tuple(pp["V"].reshape(-1).tolist()))
    if key not in _NC_CACHE:
        _NC_CACHE[key] = _build_nc(*key)
    nc = _NC_CACHE[key]

    in_maps = _make_inputs(x, W1, b1, W2, b2, pp)
    res = run_bass_kernel_spmd(nc, in_maps, list(range(NCORES)), trace=_trace)
    out = np.concatenate([res.results[c]["out2"] for c in range(NCORES)], axis=0)
    if _trace:
        kernel._last_result = res
    return np.ascontiguousarray(out[:N])
